# revision 1
# baseline (speedup 1.0000x reference)
import sys

sys.path.insert(0, "/opt/trn_rl_repo")
import numpy as np
import concourse.bass as bass
import concourse.tile as tile
from concourse import bacc, mybir
from concourse.alu_op_type import AluOpType
from concourse.bass_utils import run_bass_kernel_spmd

# Problem constants (nn_EquivGNNEncoder: 2048 graphs x 32 atoms, 3 layers)
B, NA = 2048, 32
N = B * NA                  # 65536 nodes
S_MUL, V_MUL = 32, 16
NCORES = 8
GPC = B // NCORES           # 256 graphs per core
NPC = GPC * NA              # 8192 nodes per core
GPB = 4                     # graphs per block (4*32 = 128 partitions)
NBLK = GPC // GPB           # 64 blocks per core
LAT = 128                   # latent out dim
HID = 256

INV_SQRT3 = 1.0 / np.sqrt(3.0)
C_SCALAR = np.float32(1.0 / np.sqrt(48.0))
C_VECTOR = np.float32(np.sqrt(3.0 / 48.0))

F32 = mybir.dt.float32
F32R = mybir.dt.float32r
BF16 = mybir.dt.bfloat16

_CACHE = {}


def _build_program():
    nc = bacc.Bacc("TRN2", target_bir_lowering=False, debug=False)

    s0_ap = nc.dram_tensor("s0", [NPC, S_MUL], F32, kind="ExternalInput").ap()
    posT_ap = nc.dram_tensor("posT", [NBLK, 3, 128], F32, kind="ExternalInput").ap()
    posnm_ap = nc.dram_tensor("posnm", [NPC, 3], F32, kind="ExternalInput").ap()
    bd_ap = nc.dram_tensor("bd", [128, 128], F32, kind="ExternalInput").ap()
    # transform weights, per layer, partition-aligned to lhsT slices:
    # [0:32,0:32]=Wa  [0:16,32:64]=Wb  [0:32,64:80]=Wc  [32+16c:48+16c,80:96]=Wd
    wt_ap = nc.dram_tensor("wt", [3, 128, 224], F32, kind="ExternalInput").ap()
    poolm_ap = nc.dram_tensor("poolm", [128, GPB], F32, kind="ExternalInput").ap()
    wr1_ap = nc.dram_tensor("wr1", [112, HID], F32, kind="ExternalInput").ap()
    br1_ap = nc.dram_tensor("br1", [HID, 1], F32, kind="ExternalInput").ap()
    wr2_ap = nc.dram_tensor("wr2", [HID, LAT], F32, kind="ExternalInput").ap()
    br2_ap = nc.dram_tensor("br2", [LAT, 1], F32, kind="ExternalInput").ap()
    out_ap = nc.dram_tensor("outfm", [LAT, GPC], F32, kind="ExternalOutput").ap()

    with tile.TileContext(nc) as tc:
        with tc.tile_pool(name="const", bufs=1) as const, \
             tc.tile_pool(name="stage", bufs=4) as stage, \
             tc.tile_pool(name="gmp", bufs=3) as gmp, \
             tc.tile_pool(name="feat", bufs=4, space="SBUF") as featp, \
             tc.tile_pool(name="work", bufs=4) as work, \
             tc.tile_pool(name="psagg", bufs=3, space="PSUM") as psp_agg, \
             tc.tile_pool(name="psh", bufs=3, space="PSUM") as psp_h, \
             tc.tile_pool(name="pspool", bufs=2, space="PSUM") as psp_pool:

            # --- constants ---
            bd = const.tile([128, 128], F32)
            nc.sync.dma_start(bd[:], bd_ap[:])
            wts_f = const.tile([128, 3, 224], F32)
            nc.sync.dma_start(
                wts_f[:],
                bass.AP(tensor=wt_ap.tensor, offset=wt_ap.offset,
                        ap=[[224, 128], [128 * 224, 3], [1, 224]]),
            )
            wts = const.tile([128, 3, 224], F32R)
            nc.vector.tensor_copy(wts[:], wts_f[:])
            poolm_f = const.tile([128, GPB], F32)
            nc.sync.dma_start(poolm_f[:], poolm_ap[:])
            poolm = const.tile([128, GPB], F32R)
            nc.vector.tensor_copy(poolm[:], poolm_f[:])
            wr1_f = const.tile([112, HID], F32)
            nc.sync.dma_start(wr1_f[:], wr1_ap[:])
            wr1 = const.tile([112, HID], F32R)
            nc.vector.tensor_copy(wr1[:], wr1_f[:])
            wr2a_f = const.tile([128, LAT], F32)
            nc.sync.dma_start(wr2a_f[:], wr2_ap[0:128, :])
            wr2a = const.tile([128, LAT], F32R)
            nc.vector.tensor_copy(wr2a[:], wr2a_f[:])
            wr2b_f = const.tile([128, LAT], F32)
            nc.sync.dma_start(wr2b_f[:], wr2_ap[128:256, :])
            wr2b = const.tile([128, LAT], F32R)
            nc.vector.tensor_copy(wr2b[:], wr2b_f[:])
            br1a = const.tile([128, 1], F32)
            nc.sync.dma_start(br1a[:], br1_ap[0:128, :])
            br1b = const.tile([128, 1], F32)
            nc.sync.dma_start(br1b[:], br1_ap[128:256, :])
            br2 = const.tile([LAT, 1], F32)
            nc.sync.dma_start(br2[:], br2_ap[:])
            epsb = const.tile([128, 1], F32)
            nc.vector.memset(epsb[:], 1e-12)
            zer80 = const.tile([128, 80], F32)
            nc.vector.memset(zer80[:], 0.0)
            zer32 = const.tile([32, 128], F32)
            nc.vector.memset(zer32[:], 0.0)

            # pooled per-graph features, feature-major [80, 256]
            xfm = const.tile([112, GPC], F32R)

            def emit_load_gm(b):
                f_all = stage.tile([128, 384], F32, tag="fall")
                nc.sync.dma_start(
                    f_all[:],
                    bass.AP(tensor=posT_ap.tensor, offset=posT_ap.offset + b * 3 * 128,
                            ap=[[0, 128], [128, 3], [1, 128]]),
                )
                pos_blk = stage.tile([128, 3], F32, tag="posb")
                nc.sync.dma_start(pos_blk[:], posnm_ap[b * 128:(b + 1) * 128, :])
                s0_blk = stage.tile([128, S_MUL], F32, tag="s0b")
                nc.sync.dma_start(s0_blk[:], s0_ap[b * 128:(b + 1) * 128, :])

                p_all = work.tile([128, 384], F32, tag="pall")
                nc.gpsimd.tensor_copy(
                    p_all[:],
                    bass.AP(tensor=pos_blk.tensor, offset=pos_blk.offset,
                            ap=[[3, 128], [1, 3], [0, 128]]),
                )
                diff = work.tile([128, 384], F32, tag="diff")
                nc.vector.tensor_sub(diff[:], f_all[:], p_all[:])
                sq = work.tile([128, 384], F32, tag="sq")
                nc.vector.tensor_mul(sq[:], diff[:], diff[:])
                d2 = work.tile([128, 128], F32, tag="d2")
                nc.vector.tensor_add(d2[:], sq[:, 0:128], sq[:, 128:256])
                nc.vector.tensor_add(d2[:], d2[:], sq[:, 256:384])

                gm = gmp.tile([128, 512], F32R, tag="gm")
                m2bd = work.tile([128, 128], F32, tag="m2bd")
                nc.vector.scalar_tensor_tensor(
                    m2bd[:], d2[:], 0.0, bd[:], AluOpType.is_gt, AluOpType.mult)
                nc.vector.scalar_tensor_tensor(
                    gm[:, 0:128], d2[:], 25.0, m2bd[:], AluOpType.is_le, AluOpType.mult)

                rs = work.tile([128, 128], F32, tag="rs")
                nc.scalar.activation(
                    rs[:], d2[:], mybir.ActivationFunctionType.Sqrt,
                    bias=epsb[:], scale=float(1.0 / 3.0))
                nc.vector.reciprocal(rs[:], rs[:])
                ga = work.tile([128, 128], F32, tag="ga")
                nc.vector.tensor_mul(ga[:], rs[:], gm[:, 0:128].bitcast(F32))
                ga3 = work.tile([128, 384], F32, tag="ga3")
                nc.gpsimd.tensor_copy(
                    ga3[:],
                    bass.AP(tensor=ga.tensor, offset=ga.offset,
                            ap=[[128, 128], [0, 3], [1, 128]]),
                )
                nc.vector.tensor_mul(gm[:, 128:512], diff[:], ga3[:])

                feat = featp.tile([128, 112], F32R, tag="feat")
                nc.vector.tensor_copy(feat[:, 0:S_MUL], s0_blk[:])
                nc.vector.tensor_copy(feat[:, S_MUL:112], zer80[:])
                return gm, feat

            def emit_layer(l, gm, feat):
                ps_agg = psp_agg.tile([112, 512], F32, tag="agg")
                nc.tensor.matmul(ps_agg[:], feat[:], gm[:], start=True, stop=True)

                svd = work.tile([16, 128], F32R, tag="svd")
                svt = work.tile([16, 128], F32, tag="svt")
                nc.scalar.copy(svt[:], ps_agg[32:48, 128:256])
                nc.vector.tensor_add(svt[:], svt[:], ps_agg[64:80, 256:384])
                nc.vector.tensor_add(svd[:], svt[:], ps_agg[96:112, 384:512])

                agg = work.tile([32, 512], F32R, tag="aggsb")
                nc.scalar.copy(agg[:], ps_agg[0:32, :])
                avt = work.tile([16, 384], F32R, tag="avt")
                for c in range(3):
                    nc.scalar.copy(
                        avt[:, 128 * c:128 * (c + 1)],
                        ps_agg[32 + 32 * c:48 + 32 * c, 0:128])

                wl = wts[:, l, :]
                ps_h = psp_h.tile([128, 112], F32, tag="psh")
                nc.tensor.matmul(ps_h[:, 0:32], agg[0:32, 0:128],
                                 wl[0:32, 0:32], start=True, stop=False)
                nc.tensor.matmul(ps_h[:, 0:32], svd[:],
                                 wl[0:16, 32:64], start=False, stop=True)
                for c in range(3):
                    o0 = 32 + 32 * c
                    ow = 32 if c < 2 else 16
                    nc.tensor.matmul(ps_h[:, o0:o0 + ow],
                                     agg[0:32, 128 * (1 + c):128 * (2 + c)],
                                     wl[0:32, 64:64 + ow], start=True, stop=False)
                    nc.tensor.matmul(ps_h[:, o0:o0 + 16],
                                     avt[:, 128 * c:128 * (c + 1)],
                                     wl[0:16, 96:112], start=False, stop=True)

                featn = featp.tile([128, 112], F32R, tag="feat")
                nc.vector.scalar_tensor_tensor(
                    featn[:], ps_h[:], 0.0, feat[:].bitcast(F32),
                    AluOpType.max, AluOpType.add)
                return featn

            def emit_pool(b, feat):
                ps_pool = psp_pool.tile([112, GPB], F32, tag="pool")
                nc.tensor.matmul(ps_pool[:], feat[:], poolm[:], start=True, stop=True)
                nc.vector.tensor_copy(xfm[:, b * GPB:(b + 1) * GPB], ps_pool[:])

            # interleave two independent blocks at every stage so each engine
            # always has adjacent independent work to fill dependency stalls
            IW = 2
            for grp in range(NBLK // IW):
                bs = [IW * grp + i for i in range(IW)]
                st = [emit_load_gm(b) for b in bs]
                gms = [s[0] for s in st]
                fts = [s[1] for s in st]
                for l in range(3):
                    for i in range(IW):
                        fts[i] = emit_layer(l, gms[i], fts[i])
                for i in range(IW):
                    emit_pool(bs[i], fts[i])

            # --- readout MLP: relu(x @ Wr1 + br1) @ Wr2 + br2, feature-major ---
            ps_h1 = psp_h.tile([128, GPC], F32, tag="psh")
            ps_h2 = psp_h.tile([128, GPC], F32, tag="psh")
            nc.tensor.matmul(ps_h1[:], wr1[:, 0:128], xfm[:], start=True, stop=True)
            nc.tensor.matmul(ps_h2[:], wr1[:, 128:256], xfm[:], start=True, stop=True)
            hid1 = work.tile([128, GPC], F32R, tag="hid1")
            hid2 = work.tile([128, GPC], F32R, tag="hid2")
            nc.vector.tensor_scalar(hid1[:], ps_h1[:], br1a[:], 0.0,
                                    AluOpType.add, AluOpType.max)
            nc.vector.tensor_scalar(hid2[:], ps_h2[:], br1b[:], 0.0,
                                    AluOpType.add, AluOpType.max)
            ps_o = psp_agg.tile([LAT, GPC], F32, tag="agg")
            nc.tensor.matmul(ps_o[:], wr2a[:], hid1[:], start=True, stop=False)
            nc.tensor.matmul(ps_o[:], wr2b[:], hid2[:], start=False, stop=True)
            outt = work.tile([LAT, GPC], F32, tag="outt")
            nc.vector.tensor_scalar(outt[:], ps_o[:], br2[:], None, AluOpType.add)
            nc.sync.dma_start(out_ap[:], outt[:])

    nc.compile()
    return nc


def kernel(pos, emb, W_s2n, W1, W2, W3, W4, Ws, Wv, Wr1, br1, Wr2, br2,
           z, batch, edge_index, num_graphs):
    pos = np.asarray(pos, dtype=np.float32)
    z = np.asarray(z)
    emb = np.asarray(emb, dtype=np.float32)
    W_s2n = np.asarray(W_s2n, dtype=np.float32)
    W1 = np.asarray(W1, dtype=np.float32); W2 = np.asarray(W2, dtype=np.float32)
    W3 = np.asarray(W3, dtype=np.float32); W4 = np.asarray(W4, dtype=np.float32)
    Ws = np.asarray(Ws, dtype=np.float32); Wv = np.asarray(Wv, dtype=np.float32)
    Wr1 = np.asarray(Wr1, dtype=np.float32); br1 = np.asarray(br1, dtype=np.float32)
    Wr2 = np.asarray(Wr2, dtype=np.float32); br2 = np.asarray(br2, dtype=np.float32)

    # host prep: embedding lookup folded with input linear
    EW = (emb @ W_s2n) * np.float32(1.0 / np.sqrt(S_MUL))     # [100, 32]
    s0 = EW[z]                                                # [N, 32]

    # transform weights with norm constants folded in
    wt = np.zeros((3, 128, 224), np.float32)
    cs = C_SCALAR * np.float32(1.0 / np.sqrt(S_MUL))
    csb = C_SCALAR * np.float32(INV_SQRT3 / np.sqrt(S_MUL))
    cv = C_VECTOR * np.float32(INV_SQRT3 / np.sqrt(V_MUL))
    for l in range(3):
        wt[l, 0:32, 0:32] = cs * (W1[l] @ Ws[l])
        wt[l, 0:16, 32:64] = csb * (W4[l] @ Ws[l])
        wt[l, 0:32, 64:80] = cv * (W2[l] @ Wv[l])
        wt[l, 0:16, 96:112] = cv * (W3[l] @ Wv[l])
        wt[l, 32:48, 96:112] = cv * (W3[l] @ Wv[l])
        wt[l, 64:80, 96:112] = cv * (W3[l] @ Wv[l])

    wr1p = np.zeros((112, HID), np.float32)
    wr1p[0:32] = Wr1[0:32]
    for c in range(3):
        for u in range(V_MUL):
            wr1p[32 + 32 * c + u] = Wr1[32 + 3 * u + c]

    bdm = np.zeros((128, 128), np.float32)
    for g in range(GPB):
        bdm[g * NA:(g + 1) * NA, g * NA:(g + 1) * NA] = 1.0
    poolm = np.zeros((128, GPB), np.float32)
    for g in range(GPB):
        poolm[g * NA:(g + 1) * NA, g] = 1.0

    if "nc" not in _CACHE:
        _CACHE["nc"] = _build_program()
    nc = _CACHE["nc"]

    in_maps = []
    for c in range(NCORES):
        psl = pos[c * NPC:(c + 1) * NPC]                       # [8192, 3]
        posT = np.ascontiguousarray(
            psl.reshape(NBLK, 128, 3).transpose(0, 2, 1))      # [64, 3, 128]
        in_maps.append(dict(
            s0=np.ascontiguousarray(s0[c * NPC:(c + 1) * NPC]),
            posT=posT,
            posnm=np.ascontiguousarray(psl),
            bd=bdm, wt=wt, poolm=poolm,
            wr1=wr1p, br1=br1.reshape(HID, 1),
            wr2=Wr2, br2=br2.reshape(LAT, 1),
        ))

    res = run_bass_kernel_spmd(nc, in_maps, core_ids=list(range(NCORES)))
    out = np.empty((B, LAT), np.float32)
    for c in range(NCORES):
        out[c * GPC:(c + 1) * GPC] = res.results[c]["outfm"].T
    return out



# revision 16
# speedup vs baseline: 2.7352x; 2.7352x over previous
import sys

sys.path.insert(0, "/opt/trn_rl_repo")
import numpy as np
import ml_dtypes
import concourse.bass as bass
import concourse.tile as tile
from concourse import bacc, mybir
from concourse.alu_op_type import AluOpType
from concourse.bass_utils import run_bass_kernel_spmd

# Problem constants (nn_EquivGNNEncoder: 2048 graphs x 32 atoms, 3 layers)
B, NA = 2048, 32
N = B * NA                  # 65536 nodes
S_MUL, V_MUL = 32, 16
NCORES = 8
GPC = B // NCORES           # 256 graphs per core
NPC = GPC * NA              # 8192 nodes per core
GPB = 4                     # graphs per block (4*32 = 128 partitions)
NBLK = GPC // GPB           # 64 blocks per core
NPAIR = NBLK // 2           # 32 block-pairs per core
GCH = 4                     # gm DMA chunks
BPCH = NBLK // GCH          # blocks per gm chunk
LAT = 128
HID = 256
# padded feature layout (partition ranges must start at multiples of 32):
# s(0:32) vx(32:48) pad(48:64) vy(64:80) pad(80:96) vz(96:112)
FD = 112

INV_SQRT3 = 1.0 / np.sqrt(3.0)
C_SCALAR = np.float32(1.0 / np.sqrt(48.0))
C_VECTOR = np.float32(np.sqrt(3.0 / 48.0))

F32 = mybir.dt.float32
BF16 = mybir.dt.bfloat16
BF16NP = ml_dtypes.bfloat16

_CACHE = {}


def _rf(apx, dims):
    """Return a copy of AP with the free dims replaced (partition dim kept)."""
    return bass.AP(tensor=apx.tensor, offset=apx.offset,
                   ap=[list(apx.ap[0])] + [list(d) for d in dims])


def _build_program():
    nc = bacc.Bacc("TRN2", target_bir_lowering=False, debug=False)

    gm_aps = [
        nc.dram_tensor(f"gm{k}", [128, BPCH * 512], BF16, kind="ExternalInput").ap()
        for k in range(GCH)
    ]
    s0_ap = nc.dram_tensor("s0", [128, NBLK * S_MUL], BF16, kind="ExternalInput").ap()
    s0w_ap = nc.dram_tensor("s0w", [128, NBLK * 48], BF16, kind="ExternalInput").ap()
    wta_ap = nc.dram_tensor("wta", [FD, 2 * FD], BF16, kind="ExternalInput").ap()
    wtb_ap = nc.dram_tensor("wtb", [FD, 2 * FD], BF16, kind="ExternalInput").ap()
    poolm_ap = nc.dram_tensor("poolm", [128, GPB], BF16, kind="ExternalInput").ap()
    wr1_ap = nc.dram_tensor("wr1", [FD, HID], BF16, kind="ExternalInput").ap()
    wr2_ap = nc.dram_tensor("wr2", [128, HID], BF16, kind="ExternalInput").ap()
    br1_ap = nc.dram_tensor("br1", [128, 2], F32, kind="ExternalInput").ap()
    br2_ap = nc.dram_tensor("br2", [128, 1], F32, kind="ExternalInput").ap()
    out_ap = nc.dram_tensor("outfm", [LAT, GPC], F32, kind="ExternalOutput").ap()

    with tile.TileContext(nc) as tc:
        with tc.tile_pool(name="const", bufs=1) as const, \
             tc.tile_pool(name="feat", bufs=6) as featp, \
             tc.tile_pool(name="work", bufs=3) as work, \
             tc.tile_pool(name="outp", bufs=1) as outp, \
             tc.tile_pool(name="psagg", bufs=2, space="PSUM") as pp_agg, \
             tc.tile_pool(name="psh", bufs=2, space="PSUM") as pp_h, \
             tc.tile_pool(name="pspool", bufs=1, space="PSUM") as pp_pool:

            # --- resident inputs + constants ---
            gm_ts = []
            for k in range(GCH):
                g = const.tile([128, BPCH * 512], BF16)
                nc.sync.dma_start(g[:], gm_aps[k][:])
                gm_ts.append(g)
            s0_t = const.tile([128, NBLK * S_MUL], BF16)
            nc.sync.dma_start(s0_t[:], s0_ap[:])
            s0w_t = const.tile([128, NBLK * 48], BF16)
            nc.sync.dma_start(s0w_t[:], s0w_ap[:])
            wta_t = const.tile([FD, 2 * FD], BF16)
            nc.sync.dma_start(wta_t[:], wta_ap[:])
            wtb_t = const.tile([FD, 2 * FD], BF16)
            nc.sync.dma_start(wtb_t[:], wtb_ap[:])
            poolm_t = const.tile([128, GPB], BF16)
            nc.sync.dma_start(poolm_t[:], poolm_ap[:])
            wr1_t = const.tile([FD, HID], BF16)
            nc.sync.dma_start(wr1_t[:], wr1_ap[:])
            wr2_t = const.tile([128, HID], BF16)
            nc.sync.dma_start(wr2_t[:], wr2_ap[:])
            br1_t = const.tile([128, 2], F32)
            nc.sync.dma_start(br1_t[:], br1_ap[:])
            br2_t = const.tile([128, 1], F32)
            nc.sync.dma_start(br2_t[:], br2_ap[:])

            ps_pool = pp_pool.tile([FD, GPC], F32)

            def gm_rhs(b):
                c0 = (b % BPCH) * 512
                return gm_ts[b // BPCH][:, c0:c0 + 512]

            def gm_blk(b, k):
                c0 = (b % BPCH) * 512 + 128 * k
                return gm_ts[b // BPCH][:, c0:c0 + 128]

            def pair_src(ps, r0, r1, cbase):
                # rows r0:r1 of ps_agg, columns {cbase..+128} and {cbase+512..}
                return _rf(ps[r0:r1, cbase:cbase + 128], [[512, 2], [1, 128]])

            def pair_dst(sl):
                return _rf(sl, [[128, 2], [1, 128]])

            def emit_l1(t):
                # transform-first layer 1: s0W precomputed on host, so each
                # output group is one adjacency^T @ s0W matmul, no copies
                ps_h = pp_h.tile([128, 2 * FD], F32, tag="psh")
                for h in (0, 1):
                    b = 2 * t + h
                    sa = s0w_t[:, 48 * b:48 * b + 32]
                    sc = s0w_t[:, 48 * b + 32:48 * b + 48]
                    o = FD * h
                    nc.tensor.matmul(ps_h[:, o:o + 32], gm_blk(b, 0), sa,
                                     start=True, stop=True)
                    for k in range(3):
                        nc.tensor.matmul(
                            ps_h[:, o + 32 * (k + 1):o + 32 * (k + 1) + 16],
                            gm_blk(b, 1 + k), sc, start=True, stop=True)
                featn = featp.tile([128, 2 * FD], BF16, tag="feat")
                s0pair = _rf(s0_t[:, 64 * t:64 * t + 64], [[32, 2], [1, 32]])
                nc.vector.scalar_tensor_tensor(
                    _rf(featn[:, 0:32], [[FD, 2], [1, 32]]),
                    _rf(ps_h[:, 0:32], [[FD, 2], [1, 32]]),
                    0.0, s0pair, AluOpType.max, AluOpType.add)
                nc.scalar.activation(
                    _rf(featn[:, 32:48], [[FD, 2], [32, 3], [1, 16]]),
                    _rf(ps_h[:, 32:48], [[FD, 2], [32, 3], [1, 16]]),
                    mybir.ActivationFunctionType.Relu)
                nc.vector.memset(
                    _rf(featn[:, 48:64], [[FD, 2], [32, 2], [1, 16]]), 0.0)
                return featn

            def emit_layer(l, t, feat2):
                ps_agg = pp_agg.tile([FD, 1024], F32, tag="agg")
                for h in (0, 1):
                    b = 2 * t + h
                    nc.tensor.matmul(
                        ps_agg[0:FD, 512 * h:512 * h + 512],
                        feat2[:, FD * h:FD * h + FD], gm_rhs(b),
                        start=True, stop=True)
                # ta: all features aggregated with plain adjacency A (pads = 0)
                ta = work.tile([FD, 256], BF16, tag="ta")
                nc.vector.tensor_copy(pair_dst(ta[:, :]), pair_src(ps_agg, 0, FD, 0))
                # stb rows: s@Ax(0:32) s@Ay(32:64) s@Az(64:96) svd(96:112)
                stb = work.tile([FD, 256], BF16, tag="stb")
                nc.scalar.copy(pair_dst(stb[0:32, :]), pair_src(ps_agg, 0, 32, 128))
                nc.vector.tensor_copy(pair_dst(stb[32:64, :]),
                                      pair_src(ps_agg, 0, 32, 256))
                nc.scalar.copy(pair_dst(stb[64:96, :]), pair_src(ps_agg, 0, 32, 384))
                # svd = vx@Ax + vy@Ay + vz@Az (one PSUM operand per op)
                tmpa = work.tile([16, 256], F32, tag="tmpa")
                nc.scalar.copy(pair_dst(tmpa[:, :]), pair_src(ps_agg, 32, 48, 128))
                tmpb = work.tile([16, 256], F32, tag="tmpb")
                nc.vector.tensor_add(pair_dst(tmpb[:, :]), pair_dst(tmpa[:, :]),
                                     pair_src(ps_agg, 64, 80, 256))
                nc.vector.tensor_add(pair_dst(stb[96:112, :]), pair_dst(tmpb[:, :]),
                                     pair_src(ps_agg, 96, 112, 384))
                ps_h = pp_h.tile([128, 2 * FD], F32, tag="psh")
                wl_a = wta_t[:, FD * (l - 1):FD * l]
                wl_b = wtb_t[:, FD * (l - 1):FD * l]
                for h in (0, 1):
                    nc.tensor.matmul(ps_h[:, FD * h:FD * h + FD],
                                     ta[:, 128 * h:128 * h + 128], wl_a,
                                     start=True, stop=False)
                    nc.tensor.matmul(ps_h[:, FD * h:FD * h + FD],
                                     stb[:, 128 * h:128 * h + 128], wl_b,
                                     start=False, stop=True)
                relu_t = work.tile([128, 2 * FD], BF16, tag="rel")
                nc.vector.tensor_scalar_max(relu_t[:], ps_h[:], 0.0)
                featn = featp.tile([128, 2 * FD], BF16, tag="feat")
                nc.gpsimd.tensor_add(featn[:], relu_t[:], feat2[:])
                return featn

            def emit_pool(t, feat2):
                for h in (0, 1):
                    b = 2 * t + h
                    nc.tensor.matmul(ps_pool[0:FD, 4 * b:4 * b + 4],
                                     feat2[:, FD * h:FD * h + FD], poolm_t[:],
                                     start=True, stop=True)

            # interleave two pairs (4 blocks) at every stage
            for grp in range(NPAIR // 2):
                ts = [2 * grp, 2 * grp + 1]
                fts = [emit_l1(t) for t in ts]
                for l in (1, 2):
                    fts = [emit_layer(l, ts[i], fts[i]) for i in range(2)]
                for i in range(2):
                    emit_pool(ts[i], fts[i])

            # --- readout MLP: relu(x @ Wr1 + br1) @ Wr2 + br2, feature-major ---
            xfm = outp.tile([FD, GPC], BF16, tag="xfm")
            nc.vector.tensor_copy(xfm[:], ps_pool[:])
            ps_t1 = pp_h.tile([128, GPC], F32, tag="psh")
            ps_t2 = pp_h.tile([128, GPC], F32, tag="psh")
            nc.tensor.matmul(ps_t1[:], wr1_t[:, 0:128], xfm[:], start=True, stop=True)
            nc.tensor.matmul(ps_t2[:], wr1_t[:, 128:256], xfm[:], start=True, stop=True)
            hid1 = outp.tile([128, GPC], BF16, tag="hid1")
            hid2 = outp.tile([128, GPC], BF16, tag="hid2")
            nc.vector.tensor_scalar(hid1[:], ps_t1[:], br1_t[:, 0:1], 0.0,
                                    AluOpType.add, AluOpType.max)
            nc.vector.tensor_scalar(hid2[:], ps_t2[:], br1_t[:, 1:2], 0.0,
                                    AluOpType.add, AluOpType.max)
            ps_o = pp_agg.tile([LAT, GPC], F32, tag="agg")
            nc.tensor.matmul(ps_o[:], wr2_t[:, 0:128], hid1[:], start=True, stop=False)
            nc.tensor.matmul(ps_o[:], wr2_t[:, 128:256], hid2[:], start=False, stop=True)
            out_sb = outp.tile([LAT, GPC], F32, tag="out")
            nc.vector.tensor_scalar(out_sb[:], ps_o[:], br2_t[:], None, AluOpType.add)
            nc.sync.dma_start(out_ap[:], out_sb[:])

    nc.compile()
    return nc


def kernel(pos, emb, W_s2n, W1, W2, W3, W4, Ws, Wv, Wr1, br1, Wr2, br2,
           z, batch, edge_index, num_graphs):
    pos = np.asarray(pos, dtype=np.float32)
    z = np.asarray(z)
    emb = np.asarray(emb, dtype=np.float32)
    W_s2n = np.asarray(W_s2n, dtype=np.float32)
    W1 = np.asarray(W1, dtype=np.float32); W2 = np.asarray(W2, dtype=np.float32)
    W3 = np.asarray(W3, dtype=np.float32); W4 = np.asarray(W4, dtype=np.float32)
    Ws = np.asarray(Ws, dtype=np.float32); Wv = np.asarray(Wv, dtype=np.float32)
    Wr1 = np.asarray(Wr1, dtype=np.float32); br1 = np.asarray(br1, dtype=np.float32)
    Wr2 = np.asarray(Wr2, dtype=np.float32); br2 = np.asarray(br2, dtype=np.float32)

    # host prep: embedding lookup folded with input linear
    EW = (emb @ W_s2n) * np.float32(1.0 / np.sqrt(S_MUL))     # [100, 32]
    s0 = EW[z]                                                # [N, 32]

    # masked adjacency + spherical harmonics: gm[b, src, (type, dst)]
    pos_g = pos.reshape(B, NA, 3)
    diff = pos_g[:, None, :, :] - pos_g[:, :, None, :]        # [B, s, d, c] = pos[d]-pos[s]
    d2 = (diff * diff).sum(-1)
    mask = ((d2 <= 25.0) & (d2 > 0.0)).astype(np.float32)
    with np.errstate(divide="ignore", invalid="ignore"):
        inv_r = np.float32(np.sqrt(3.0)) / np.sqrt(d2)
    inv_r[~np.isfinite(inv_r)] = 0.0
    sh = diff * (mask * inv_r)[..., None]                     # [B, s, d, 3]
    NB4 = B // GPB
    Tall = np.empty((4, NB4, GPB, NA, NA), np.float32)
    Tall[0] = mask.reshape(NB4, GPB, NA, NA)
    for c in range(3):
        Tall[1 + c] = sh[..., c].reshape(NB4, GPB, NA, NA)
    TT = np.zeros((NB4, GPB, NA, 4, GPB, NA), np.float32)
    for g in range(GPB):
        TT[:, g, :, :, g, :] = Tall[:, :, g].transpose(1, 2, 0, 3)
    gm_all = TT.reshape(NB4, 128, 512)

    # folded tensor-product + linear weights
    cs = C_SCALAR * np.float32(1.0 / np.sqrt(S_MUL))
    csb = C_SCALAR * np.float32(INV_SQRT3 / np.sqrt(S_MUL))
    cv = C_VECTOR * np.float32(INV_SQRT3 / np.sqrt(V_MUL))
    Wa = [cs * (W1[l] @ Ws[l]) for l in range(3)]     # [32, 32]
    Wb = [csb * (W4[l] @ Ws[l]) for l in range(3)]    # [16, 32]
    Wc = [cv * (W2[l] @ Wv[l]) for l in range(3)]     # [32, 16]
    Wd = [cv * (W3[l] @ Wv[l]) for l in range(3)]     # [16, 16]

    # layer-1 transform applied on host (s0 is host-prepped anyway)
    s0w = np.concatenate([s0 @ Wa[0], s0 @ Wc[0]], axis=1)    # [N, 48]

    # feature/psum row layout: s(0:32) vx(32:48) pad vy(64:80) pad vz(96:112)
    # stb rows: [s@Ax(0:32), s@Ay(32:64), s@Az(64:96), svd(96:112)]
    wta = np.zeros((FD, 2 * FD), np.float32)
    wtb = np.zeros((FD, 2 * FD), np.float32)
    for l in (1, 2):
        o = FD * (l - 1)
        wta[0:32, o:o + 32] = Wa[l]
        for c in range(3):
            r = 32 * (c + 1)
            wta[r:r + 16, o + r:o + r + 16] = Wd[l]
        for c in range(3):
            wtb[32 * c:32 * c + 32, o + 32 * (c + 1):o + 32 * (c + 1) + 16] = Wc[l]
        wtb[96:112, o:o + 32] = Wb[l]

    poolm = np.zeros((128, GPB), np.float32)
    for g in range(GPB):
        poolm[g * NA:(g + 1) * NA, g] = 1.0

    # readout weights: v rows at 32*(c+1)+u map to original 32+3u+c
    wr1p = np.zeros((FD, HID), np.float32)
    wr1p[0:32] = Wr1[0:32]
    for c in range(3):
        for u in range(V_MUL):
            wr1p[32 * (c + 1) + u] = Wr1[32 + 3 * u + c]
    wr2p = np.zeros((128, HID), np.float32)
    wr2p[:, 0:128] = Wr2[0:128]
    wr2p[:, 128:256] = Wr2[128:256]
    br1t = br1.reshape(2, 128).T.copy()               # [128, 2]
    br2t = br2.reshape(LAT, 1)

    if "nc" not in _CACHE:
        _CACHE["nc"] = _build_program()
    nc = _CACHE["nc"]

    consts = dict(
        wta=wta.astype(BF16NP), wtb=wtb.astype(BF16NP),
        poolm=poolm.astype(BF16NP), wr1=wr1p.astype(BF16NP), wr2=wr2p.astype(BF16NP),
        br1=np.ascontiguousarray(br1t), br2=br2t,
    )
    in_maps = []
    for c in range(NCORES):
        gm_core = np.ascontiguousarray(
            gm_all[c * NBLK:(c + 1) * NBLK].transpose(1, 0, 2)
        ).reshape(128, NBLK * 512).astype(BF16NP)
        s0_core = np.ascontiguousarray(
            s0[c * NPC:(c + 1) * NPC].reshape(NBLK, 128, S_MUL).transpose(1, 0, 2)
        ).reshape(128, NBLK * S_MUL).astype(BF16NP)
        s0w_core = np.ascontiguousarray(
            s0w[c * NPC:(c + 1) * NPC].reshape(NBLK, 128, 48).transpose(1, 0, 2)
        ).reshape(128, NBLK * 48).astype(BF16NP)
        m = dict(consts)
        for k in range(GCH):
            m[f"gm{k}"] = np.ascontiguousarray(
                gm_core[:, k * BPCH * 512:(k + 1) * BPCH * 512])
        m["s0"] = s0_core
        m["s0w"] = s0w_core
        in_maps.append(m)

    res = run_bass_kernel_spmd(nc, in_maps, core_ids=list(range(NCORES)))
    out = np.empty((B, LAT), np.float32)
    for c in range(NCORES):
        out[c * GPC:(c + 1) * GPC] = res.results[c]["outfm"].T
    return out


# revision 19
# speedup vs baseline: 3.0081x; 1.0998x over previous
import sys

sys.path.insert(0, "/opt/trn_rl_repo")
import numpy as np
import ml_dtypes
import concourse.bass as bass
import concourse.tile as tile
from concourse import bacc, mybir
from concourse.alu_op_type import AluOpType
from concourse.bass_utils import run_bass_kernel_spmd

# Problem constants (nn_EquivGNNEncoder: 2048 graphs x 32 atoms, 3 layers)
B, NA = 2048, 32
N = B * NA                  # 65536 nodes
S_MUL, V_MUL = 32, 16
NCORES = 8
GPC = B // NCORES           # 256 graphs per core
NPC = GPC * NA              # 8192 nodes per core
GPB = 4                     # graphs per block (4*32 = 128 partitions)
NBLK = GPC // GPB           # 64 blocks per core
NPAIR = NBLK // 2           # 32 block-pairs per core
GCH = 4                     # gm DMA chunks
BPCH = NBLK // GCH          # blocks per gm chunk
LAT = 128
HID = 256
# padded feature layout (partition ranges must start at multiples of 32):
# s(0:32) vx(32:48) pad(48:64) vy(64:80) pad(80:96) vz(96:112)
FD = 112

INV_SQRT3 = 1.0 / np.sqrt(3.0)
C_SCALAR = np.float32(1.0 / np.sqrt(48.0))
C_VECTOR = np.float32(np.sqrt(3.0 / 48.0))

F32 = mybir.dt.float32
BF16 = mybir.dt.bfloat16
BF16NP = ml_dtypes.bfloat16

_CACHE = {}


def _rf(apx, dims):
    """Return a copy of AP with the free dims replaced (partition dim kept)."""
    return bass.AP(tensor=apx.tensor, offset=apx.offset,
                   ap=[list(apx.ap[0])] + [list(d) for d in dims])


def _build_program():
    nc = bacc.Bacc("TRN2", target_bir_lowering=False, debug=False)

    gm_aps = [
        nc.dram_tensor(f"gm{k}", [128, BPCH * 512], BF16, kind="ExternalInput").ap()
        for k in range(GCH)
    ]
    s0_ap = nc.dram_tensor("s0", [128, NBLK * S_MUL], BF16, kind="ExternalInput").ap()
    s0w_ap = nc.dram_tensor("s0w", [128, NBLK * 48], BF16, kind="ExternalInput").ap()
    wta_ap = nc.dram_tensor("wta", [FD, 2 * FD], BF16, kind="ExternalInput").ap()
    wtb_ap = nc.dram_tensor("wtb", [FD, 2 * FD], BF16, kind="ExternalInput").ap()
    poolm_ap = nc.dram_tensor("poolm", [128, GPB], BF16, kind="ExternalInput").ap()
    wr1_ap = nc.dram_tensor("wr1", [FD, HID], BF16, kind="ExternalInput").ap()
    wr2_ap = nc.dram_tensor("wr2", [128, HID], BF16, kind="ExternalInput").ap()
    br1_ap = nc.dram_tensor("br1", [128, 2], F32, kind="ExternalInput").ap()
    br2_ap = nc.dram_tensor("br2", [128, 1], F32, kind="ExternalInput").ap()
    out_ap = nc.dram_tensor("outfm", [LAT, GPC], F32, kind="ExternalOutput").ap()

    with tile.TileContext(nc) as tc:
        with tc.tile_pool(name="const", bufs=1) as const, \
             tc.tile_pool(name="feat", bufs=8) as featp, \
             tc.tile_pool(name="work", bufs=4) as work, \
             tc.tile_pool(name="outp", bufs=1) as outp, \
             tc.tile_pool(name="psagg", bufs=2, space="PSUM") as pp_agg, \
             tc.tile_pool(name="psh", bufs=3, space="PSUM") as pp_h, \
             tc.tile_pool(name="pspool", bufs=1, space="PSUM") as pp_pool:

            # --- resident inputs + constants ---
            gm_ts = []
            for k in range(GCH):
                g = const.tile([128, BPCH * 512], BF16, tag=f"gm{k}")
                nc.sync.dma_start(g[:], gm_aps[k][:])
                gm_ts.append(g)
            s0_t = const.tile([128, NBLK * S_MUL], BF16)
            nc.sync.dma_start(s0_t[:], s0_ap[:])
            s0w_t = const.tile([128, NBLK * 48], BF16)
            nc.sync.dma_start(s0w_t[:], s0w_ap[:])
            wta_t = const.tile([FD, 2 * FD], BF16)
            nc.sync.dma_start(wta_t[:], wta_ap[:])
            wtb_t = const.tile([FD, 2 * FD], BF16)
            nc.sync.dma_start(wtb_t[:], wtb_ap[:])
            poolm_t = const.tile([128, GPB], BF16)
            nc.sync.dma_start(poolm_t[:], poolm_ap[:])
            wr1_t = const.tile([FD, HID], BF16)
            nc.sync.dma_start(wr1_t[:], wr1_ap[:])
            wr2_t = const.tile([128, HID], BF16)
            nc.sync.dma_start(wr2_t[:], wr2_ap[:])
            br1_t = const.tile([128, 2], F32)
            nc.sync.dma_start(br1_t[:], br1_ap[:])
            br2_t = const.tile([128, 1], F32)
            nc.sync.dma_start(br2_t[:], br2_ap[:])

            ps_pool = pp_pool.tile([FD, GPC], F32)

            def gm_rhs(b):
                c0 = (b % BPCH) * 512
                return gm_ts[b // BPCH][:, c0:c0 + 512]

            def gm_blk(b, k):
                c0 = (b % BPCH) * 512 + 128 * k
                return gm_ts[b // BPCH][:, c0:c0 + 128]

            def pair_src(ps, r0, r1, cbase):
                # rows r0:r1 of ps_agg, columns {cbase..+128} and {cbase+512..}
                return _rf(ps[r0:r1, cbase:cbase + 128], [[512, 2], [1, 128]])

            def pair_dst(sl):
                return _rf(sl, [[128, 2], [1, 128]])

            def emit_l1(t):
                # transform-first layer 1: s0W precomputed on host, so each
                # output group is one adjacency^T @ s0W matmul, no copies
                ps_h = pp_h.tile([128, 2 * FD], F32, tag="psh")
                for h in (0, 1):
                    b = 2 * t + h
                    sa = s0w_t[:, 48 * b:48 * b + 32]
                    sc = s0w_t[:, 48 * b + 32:48 * b + 48]
                    o = FD * h
                    nc.tensor.matmul(ps_h[:, o:o + 32], gm_blk(b, 0), sa,
                                     start=True, stop=True)
                    for k in range(3):
                        nc.tensor.matmul(
                            ps_h[:, o + 32 * (k + 1):o + 32 * (k + 1) + 16],
                            gm_blk(b, 1 + k), sc, start=True, stop=True)
                featn = featp.tile([128, 2 * FD], BF16, tag="feat")
                s0pair = _rf(s0_t[:, 64 * t:64 * t + 64], [[32, 2], [1, 32]])
                nc.vector.scalar_tensor_tensor(
                    _rf(featn[:, 0:32], [[FD, 2], [1, 32]]),
                    _rf(ps_h[:, 0:32], [[FD, 2], [1, 32]]),
                    0.0, s0pair, AluOpType.max, AluOpType.add)
                nc.scalar.activation(
                    _rf(featn[:, 32:48], [[FD, 2], [32, 3], [1, 16]]),
                    _rf(ps_h[:, 32:48], [[FD, 2], [32, 3], [1, 16]]),
                    mybir.ActivationFunctionType.Relu)
                nc.gpsimd.memset(
                    _rf(featn[:, 48:64], [[FD, 2], [32, 2], [1, 16]]), 0.0)
                return featn

            def emit_layer(l, t, feat2):
                ps_agg = pp_agg.tile([FD, 1024], F32, tag="agg")
                for h in (0, 1):
                    b = 2 * t + h
                    nc.tensor.matmul(
                        ps_agg[0:FD, 512 * h:512 * h + 512],
                        feat2[:, FD * h:FD * h + FD], gm_rhs(b),
                        start=True, stop=True)
                # ta: all features aggregated with plain adjacency A (pads = 0)
                ta = work.tile([FD, 256], BF16, tag="ta")
                nc.vector.tensor_copy(pair_dst(ta[:, :]), pair_src(ps_agg, 0, FD, 0))
                # stb rows: s@Ax(0:32) s@Ay(32:64) s@Az(64:96) svd(96:112)
                stb = work.tile([FD, 256], BF16, tag="stb")
                nc.scalar.copy(pair_dst(stb[0:32, :]), pair_src(ps_agg, 0, 32, 128))
                nc.vector.tensor_copy(pair_dst(stb[32:64, :]),
                                      pair_src(ps_agg, 0, 32, 256))
                nc.scalar.copy(pair_dst(stb[64:96, :]), pair_src(ps_agg, 0, 32, 384))
                # svd = vx@Ax + vy@Ay + vz@Az (one PSUM operand per op)
                tmpa = work.tile([16, 256], F32, tag="tmpa")
                nc.scalar.copy(pair_dst(tmpa[:, :]), pair_src(ps_agg, 32, 48, 128))
                tmpb = work.tile([16, 256], F32, tag="tmpb")
                nc.vector.tensor_add(pair_dst(tmpb[:, :]), pair_dst(tmpa[:, :]),
                                     pair_src(ps_agg, 64, 80, 256))
                nc.vector.tensor_add(pair_dst(stb[96:112, :]), pair_dst(tmpb[:, :]),
                                     pair_src(ps_agg, 96, 112, 384))
                ps_h = pp_h.tile([128, 2 * FD], F32, tag="psh")
                wl_a = wta_t[:, FD * (l - 1):FD * l]
                wl_b = wtb_t[:, FD * (l - 1):FD * l]
                for h in (0, 1):
                    nc.tensor.matmul(ps_h[:, FD * h:FD * h + FD],
                                     ta[:, 128 * h:128 * h + 128], wl_a,
                                     start=True, stop=False)
                    nc.tensor.matmul(ps_h[:, FD * h:FD * h + FD],
                                     stb[:, 128 * h:128 * h + 128], wl_b,
                                     start=False, stop=True)
                relu_t = work.tile([128, 2 * FD], BF16, tag="rel")
                nc.scalar.activation(relu_t[:], ps_h[:],
                                     mybir.ActivationFunctionType.Relu)
                featn = featp.tile([128, 2 * FD], BF16, tag="feat")
                nc.gpsimd.tensor_add(featn[:], relu_t[:], feat2[:])
                return featn

            def emit_pool(t, feat2):
                for h in (0, 1):
                    b = 2 * t + h
                    nc.tensor.matmul(ps_pool[0:FD, 4 * b:4 * b + 4],
                                     feat2[:, FD * h:FD * h + FD], poolm_t[:],
                                     start=True, stop=True)

            # interleave IW pairs at every stage to hide dependency latency
            IW = 3
            t0 = 0
            while t0 < NPAIR:
                ts = list(range(t0, min(t0 + IW, NPAIR)))
                t0 += IW
                fts = [emit_l1(t) for t in ts]
                for l in (1, 2):
                    fts = [emit_layer(l, ts[i], fts[i]) for i in range(len(ts))]
                for i in range(len(ts)):
                    emit_pool(ts[i], fts[i])

            # --- readout MLP: relu(x @ Wr1 + br1) @ Wr2 + br2, feature-major ---
            xfm = outp.tile([FD, GPC], BF16, tag="xfm")
            nc.vector.tensor_copy(xfm[:], ps_pool[:])
            ps_t1 = pp_h.tile([128, GPC], F32, tag="psh")
            ps_t2 = pp_h.tile([128, GPC], F32, tag="psh")
            nc.tensor.matmul(ps_t1[:], wr1_t[:, 0:128], xfm[:], start=True, stop=True)
            nc.tensor.matmul(ps_t2[:], wr1_t[:, 128:256], xfm[:], start=True, stop=True)
            hid1 = outp.tile([128, GPC], BF16, tag="hid1")
            hid2 = outp.tile([128, GPC], BF16, tag="hid2")
            nc.vector.tensor_scalar(hid1[:], ps_t1[:], br1_t[:, 0:1], 0.0,
                                    AluOpType.add, AluOpType.max)
            nc.vector.tensor_scalar(hid2[:], ps_t2[:], br1_t[:, 1:2], 0.0,
                                    AluOpType.add, AluOpType.max)
            ps_o = pp_agg.tile([LAT, GPC], F32, tag="agg")
            nc.tensor.matmul(ps_o[:], wr2_t[:, 0:128], hid1[:], start=True, stop=False)
            nc.tensor.matmul(ps_o[:], wr2_t[:, 128:256], hid2[:], start=False, stop=True)
            out_sb = outp.tile([LAT, GPC], F32, tag="out")
            nc.vector.tensor_scalar(out_sb[:], ps_o[:], br2_t[:], None, AluOpType.add)
            nc.sync.dma_start(out_ap[:], out_sb[:])

    nc.compile()
    return nc


def kernel(pos, emb, W_s2n, W1, W2, W3, W4, Ws, Wv, Wr1, br1, Wr2, br2,
           z, batch, edge_index, num_graphs):
    pos = np.asarray(pos, dtype=np.float32)
    z = np.asarray(z)
    emb = np.asarray(emb, dtype=np.float32)
    W_s2n = np.asarray(W_s2n, dtype=np.float32)
    W1 = np.asarray(W1, dtype=np.float32); W2 = np.asarray(W2, dtype=np.float32)
    W3 = np.asarray(W3, dtype=np.float32); W4 = np.asarray(W4, dtype=np.float32)
    Ws = np.asarray(Ws, dtype=np.float32); Wv = np.asarray(Wv, dtype=np.float32)
    Wr1 = np.asarray(Wr1, dtype=np.float32); br1 = np.asarray(br1, dtype=np.float32)
    Wr2 = np.asarray(Wr2, dtype=np.float32); br2 = np.asarray(br2, dtype=np.float32)

    # host prep: embedding lookup folded with input linear
    EW = (emb @ W_s2n) * np.float32(1.0 / np.sqrt(S_MUL))     # [100, 32]
    s0 = EW[z]                                                # [N, 32]

    # masked adjacency + spherical harmonics: gm[b, src, (type, dst)]
    pos_g = pos.reshape(B, NA, 3)
    diff = pos_g[:, None, :, :] - pos_g[:, :, None, :]        # [B, s, d, c] = pos[d]-pos[s]
    d2 = (diff * diff).sum(-1)
    mask = ((d2 <= 25.0) & (d2 > 0.0)).astype(np.float32)
    with np.errstate(divide="ignore", invalid="ignore"):
        inv_r = np.float32(np.sqrt(3.0)) / np.sqrt(d2)
    inv_r[~np.isfinite(inv_r)] = 0.0
    sh = diff * (mask * inv_r)[..., None]                     # [B, s, d, 3]
    NB4 = B // GPB
    Tall = np.empty((4, NB4, GPB, NA, NA), np.float32)
    Tall[0] = mask.reshape(NB4, GPB, NA, NA)
    for c in range(3):
        Tall[1 + c] = sh[..., c].reshape(NB4, GPB, NA, NA)
    TT = np.zeros((NB4, GPB, NA, 4, GPB, NA), np.float32)
    for g in range(GPB):
        TT[:, g, :, :, g, :] = Tall[:, :, g].transpose(1, 2, 0, 3)
    gm_all = TT.reshape(NB4, 128, 512)

    # folded tensor-product + linear weights
    cs = C_SCALAR * np.float32(1.0 / np.sqrt(S_MUL))
    csb = C_SCALAR * np.float32(INV_SQRT3 / np.sqrt(S_MUL))
    cv = C_VECTOR * np.float32(INV_SQRT3 / np.sqrt(V_MUL))
    Wa = [cs * (W1[l] @ Ws[l]) for l in range(3)]     # [32, 32]
    Wb = [csb * (W4[l] @ Ws[l]) for l in range(3)]    # [16, 32]
    Wc = [cv * (W2[l] @ Wv[l]) for l in range(3)]     # [32, 16]
    Wd = [cv * (W3[l] @ Wv[l]) for l in range(3)]     # [16, 16]

    # layer-1 transform applied on host (s0 is host-prepped anyway)
    s0w = np.concatenate([s0 @ Wa[0], s0 @ Wc[0]], axis=1)    # [N, 48]

    # feature/psum row layout: s(0:32) vx(32:48) pad vy(64:80) pad vz(96:112)
    # stb rows: [s@Ax(0:32), s@Ay(32:64), s@Az(64:96), svd(96:112)]
    wta = np.zeros((FD, 2 * FD), np.float32)
    wtb = np.zeros((FD, 2 * FD), np.float32)
    for l in (1, 2):
        o = FD * (l - 1)
        wta[0:32, o:o + 32] = Wa[l]
        for c in range(3):
            r = 32 * (c + 1)
            wta[r:r + 16, o + r:o + r + 16] = Wd[l]
        for c in range(3):
            wtb[32 * c:32 * c + 32, o + 32 * (c + 1):o + 32 * (c + 1) + 16] = Wc[l]
        wtb[96:112, o:o + 32] = Wb[l]

    poolm = np.zeros((128, GPB), np.float32)
    for g in range(GPB):
        poolm[g * NA:(g + 1) * NA, g] = 1.0

    # readout weights: v rows at 32*(c+1)+u map to original 32+3u+c
    wr1p = np.zeros((FD, HID), np.float32)
    wr1p[0:32] = Wr1[0:32]
    for c in range(3):
        for u in range(V_MUL):
            wr1p[32 * (c + 1) + u] = Wr1[32 + 3 * u + c]
    wr2p = np.zeros((128, HID), np.float32)
    wr2p[:, 0:128] = Wr2[0:128]
    wr2p[:, 128:256] = Wr2[128:256]
    br1t = br1.reshape(2, 128).T.copy()               # [128, 2]
    br2t = br2.reshape(LAT, 1)

    if "nc" not in _CACHE:
        _CACHE["nc"] = _build_program()
    nc = _CACHE["nc"]

    consts = dict(
        wta=wta.astype(BF16NP), wtb=wtb.astype(BF16NP),
        poolm=poolm.astype(BF16NP), wr1=wr1p.astype(BF16NP), wr2=wr2p.astype(BF16NP),
        br1=np.ascontiguousarray(br1t), br2=br2t,
    )
    in_maps = []
    for c in range(NCORES):
        gm_core = np.ascontiguousarray(
            gm_all[c * NBLK:(c + 1) * NBLK].transpose(1, 0, 2)
        ).reshape(128, NBLK * 512).astype(BF16NP)
        s0_core = np.ascontiguousarray(
            s0[c * NPC:(c + 1) * NPC].reshape(NBLK, 128, S_MUL).transpose(1, 0, 2)
        ).reshape(128, NBLK * S_MUL).astype(BF16NP)
        s0w_core = np.ascontiguousarray(
            s0w[c * NPC:(c + 1) * NPC].reshape(NBLK, 128, 48).transpose(1, 0, 2)
        ).reshape(128, NBLK * 48).astype(BF16NP)
        m = dict(consts)
        for k in range(GCH):
            m[f"gm{k}"] = np.ascontiguousarray(
                gm_core[:, k * BPCH * 512:(k + 1) * BPCH * 512])
        m["s0"] = s0_core
        m["s0w"] = s0w_core
        in_maps.append(m)

    res = run_bass_kernel_spmd(nc, in_maps, core_ids=list(range(NCORES)))
    out = np.empty((B, LAT), np.float32)
    for c in range(NCORES):
        out[c * GPC:(c + 1) * GPC] = res.results[c]["outfm"].T
    return out


# revision 20
# speedup vs baseline: 3.0678x; 1.0198x over previous
import sys

sys.path.insert(0, "/opt/trn_rl_repo")
import numpy as np
import ml_dtypes
import concourse.bass as bass
import concourse.tile as tile
from concourse import bacc, mybir
from concourse.alu_op_type import AluOpType
from concourse.bass_utils import run_bass_kernel_spmd

# Problem constants (nn_EquivGNNEncoder: 2048 graphs x 32 atoms, 3 layers)
B, NA = 2048, 32
N = B * NA                  # 65536 nodes
S_MUL, V_MUL = 32, 16
NCORES = 8
GPC = B // NCORES           # 256 graphs per core
NPC = GPC * NA              # 8192 nodes per core
GPB = 4                     # graphs per block (4*32 = 128 partitions)
NBLK = GPC // GPB           # 64 blocks per core
NPAIR = NBLK // 2           # 32 block-pairs per core
GCH = 4                     # gm DMA chunks
BPCH = NBLK // GCH          # blocks per gm chunk
LAT = 128
HID = 256
# padded feature layout (partition ranges must start at multiples of 32):
# s(0:32) vx(32:48) pad(48:64) vy(64:80) pad(80:96) vz(96:112)
FD = 112

INV_SQRT3 = 1.0 / np.sqrt(3.0)
C_SCALAR = np.float32(1.0 / np.sqrt(48.0))
C_VECTOR = np.float32(np.sqrt(3.0 / 48.0))

F32 = mybir.dt.float32
BF16 = mybir.dt.bfloat16
BF16NP = ml_dtypes.bfloat16

_CACHE = {}


def _rf(apx, dims):
    """Return a copy of AP with the free dims replaced (partition dim kept)."""
    return bass.AP(tensor=apx.tensor, offset=apx.offset,
                   ap=[list(apx.ap[0])] + [list(d) for d in dims])


def _build_program():
    nc = bacc.Bacc("TRN2", target_bir_lowering=False, debug=False)

    gm_aps = [
        nc.dram_tensor(f"gm{k}", [128, BPCH * 512], BF16, kind="ExternalInput").ap()
        for k in range(GCH)
    ]
    s0_ap = nc.dram_tensor("s0", [128, NBLK * S_MUL], BF16, kind="ExternalInput").ap()
    s0w_ap = nc.dram_tensor("s0w", [128, NBLK * 48], BF16, kind="ExternalInput").ap()
    wta_ap = nc.dram_tensor("wta", [FD, 2 * FD], BF16, kind="ExternalInput").ap()
    wtb_ap = nc.dram_tensor("wtb", [FD, 2 * FD], BF16, kind="ExternalInput").ap()
    poolm_ap = nc.dram_tensor("poolm", [128, GPB], BF16, kind="ExternalInput").ap()
    wr1_ap = nc.dram_tensor("wr1", [FD, HID], BF16, kind="ExternalInput").ap()
    wr2_ap = nc.dram_tensor("wr2", [128, HID], BF16, kind="ExternalInput").ap()
    br1_ap = nc.dram_tensor("br1", [128, 2], F32, kind="ExternalInput").ap()
    br2_ap = nc.dram_tensor("br2", [128, 1], F32, kind="ExternalInput").ap()
    out_ap = nc.dram_tensor("outfm", [LAT, GPC], F32, kind="ExternalOutput").ap()

    with tile.TileContext(nc) as tc:
        with tc.tile_pool(name="const", bufs=1) as const, \
             tc.tile_pool(name="feat", bufs=8) as featp, \
             tc.tile_pool(name="work", bufs=4) as work, \
             tc.tile_pool(name="outp", bufs=1) as outp, \
             tc.tile_pool(name="psagg", bufs=3, space="PSUM") as pp_agg, \
             tc.tile_pool(name="psh", bufs=1, space="PSUM") as pp_h, \
             tc.tile_pool(name="pspool", bufs=1, space="PSUM") as pp_pool:

            # --- resident inputs + constants ---
            gm_ts = []
            for k in range(GCH):
                g = const.tile([128, BPCH * 512], BF16, tag=f"gm{k}")
                nc.sync.dma_start(g[:], gm_aps[k][:])
                gm_ts.append(g)
            s0_t = const.tile([128, NBLK * S_MUL], BF16)
            nc.sync.dma_start(s0_t[:], s0_ap[:])
            s0w_t = const.tile([128, NBLK * 48], BF16)
            nc.sync.dma_start(s0w_t[:], s0w_ap[:])
            wta_t = const.tile([FD, 2 * FD], BF16)
            nc.sync.dma_start(wta_t[:], wta_ap[:])
            wtb_t = const.tile([FD, 2 * FD], BF16)
            nc.sync.dma_start(wtb_t[:], wtb_ap[:])
            poolm_t = const.tile([128, GPB], BF16)
            nc.sync.dma_start(poolm_t[:], poolm_ap[:])
            wr1_t = const.tile([FD, HID], BF16)
            nc.sync.dma_start(wr1_t[:], wr1_ap[:])
            wr2_t = const.tile([128, HID], BF16)
            nc.sync.dma_start(wr2_t[:], wr2_ap[:])
            br1_t = const.tile([128, 2], F32)
            nc.sync.dma_start(br1_t[:], br1_ap[:])
            br2_t = const.tile([128, 1], F32)
            nc.sync.dma_start(br2_t[:], br2_ap[:])

            ps_pool = pp_pool.tile([FD, GPC], F32)
            # two 224-col ps_h slots packed into a single PSUM bank,
            # manually ring-buffered by pair parity
            psh_big = pp_h.tile([128, 448], F32)

            def psh_slot(t):
                o = 224 * (t % 2)
                return psh_big[:, o:o + 224]

            def gm_rhs(b):
                c0 = (b % BPCH) * 512
                return gm_ts[b // BPCH][:, c0:c0 + 512]

            def gm_blk(b, k):
                c0 = (b % BPCH) * 512 + 128 * k
                return gm_ts[b // BPCH][:, c0:c0 + 128]

            def pair_src(ps, r0, r1, cbase):
                # rows r0:r1 of ps_agg, columns {cbase..+128} and {cbase+512..}
                return _rf(ps[r0:r1, cbase:cbase + 128], [[512, 2], [1, 128]])

            def pair_dst(sl):
                return _rf(sl, [[128, 2], [1, 128]])

            def emit_l1(t):
                # transform-first layer 1: s0W precomputed on host, so each
                # output group is one adjacency^T @ s0W matmul, no copies
                po = 224 * (t % 2)
                ps_h = psh_big[:, po:po + 224]
                for h in (0, 1):
                    b = 2 * t + h
                    sa = s0w_t[:, 48 * b:48 * b + 32]
                    sc = s0w_t[:, 48 * b + 32:48 * b + 48]
                    o = po + FD * h
                    nc.tensor.matmul(psh_big[:, o:o + 32], gm_blk(b, 0), sa,
                                     start=True, stop=True)
                    for k in range(3):
                        nc.tensor.matmul(
                            psh_big[:, o + 32 * (k + 1):o + 32 * (k + 1) + 16],
                            gm_blk(b, 1 + k), sc, start=True, stop=True)
                featn = featp.tile([128, 2 * FD], BF16, tag="feat")
                s0pair = _rf(s0_t[:, 64 * t:64 * t + 64], [[32, 2], [1, 32]])
                nc.vector.memset(
                    _rf(featn[:, 48:64], [[FD, 2], [32, 2], [1, 16]]), 0.0)
                nc.vector.scalar_tensor_tensor(
                    _rf(featn[:, 0:32], [[FD, 2], [1, 32]]),
                    _rf(ps_h[:, 0:32], [[FD, 2], [1, 32]]),
                    0.0, s0pair, AluOpType.max, AluOpType.add)
                nc.scalar.activation(
                    _rf(featn[:, 32:48], [[FD, 2], [32, 3], [1, 16]]),
                    _rf(ps_h[:, 32:48], [[FD, 2], [32, 3], [1, 16]]),
                    mybir.ActivationFunctionType.Relu)
                return featn

            def emit_layer(l, t, feat2):
                ps_agg = pp_agg.tile([FD, 1024], F32, tag="agg")
                for h in (0, 1):
                    b = 2 * t + h
                    nc.tensor.matmul(
                        ps_agg[0:FD, 512 * h:512 * h + 512],
                        feat2[:, FD * h:FD * h + FD], gm_rhs(b),
                        start=True, stop=True)
                # svd chain first on Act (longest path), cA first on DVE
                tmpa = work.tile([16, 256], F32, tag="tmpa")
                nc.scalar.copy(pair_dst(tmpa[:, :]), pair_src(ps_agg, 32, 48, 128))
                # ta: all features aggregated with plain adjacency A (pads = 0)
                ta = work.tile([FD, 256], BF16, tag="ta")
                nc.vector.tensor_copy(pair_dst(ta[:, :]), pair_src(ps_agg, 0, FD, 0))
                # stb rows: s@Ax(0:32) s@Ay(32:64) s@Az(64:96) svd(96:112)
                stb = work.tile([FD, 256], BF16, tag="stb")
                nc.scalar.copy(pair_dst(stb[0:32, :]), pair_src(ps_agg, 0, 32, 128))
                nc.scalar.copy(pair_dst(stb[64:96, :]), pair_src(ps_agg, 0, 32, 384))
                tmpb = work.tile([16, 256], F32, tag="tmpb")
                nc.vector.tensor_add(pair_dst(tmpb[:, :]), pair_dst(tmpa[:, :]),
                                     pair_src(ps_agg, 64, 80, 256))
                nc.vector.tensor_add(pair_dst(stb[96:112, :]), pair_dst(tmpb[:, :]),
                                     pair_src(ps_agg, 96, 112, 384))
                nc.vector.tensor_copy(pair_dst(stb[32:64, :]),
                                      pair_src(ps_agg, 0, 32, 256))
                po = 224 * (t % 2)
                ps_h = psh_big[:, po:po + 224]
                wl_a = wta_t[:, FD * (l - 1):FD * l]
                wl_b = wtb_t[:, FD * (l - 1):FD * l]
                for h in (0, 1):
                    nc.tensor.matmul(psh_big[:, po + FD * h:po + FD * h + FD],
                                     ta[:, 128 * h:128 * h + 128], wl_a,
                                     start=True, stop=False)
                    nc.tensor.matmul(psh_big[:, po + FD * h:po + FD * h + FD],
                                     stb[:, 128 * h:128 * h + 128], wl_b,
                                     start=False, stop=True)
                featn = featp.tile([128, 2 * FD], BF16, tag="feat")
                nc.vector.scalar_tensor_tensor(
                    featn[:], ps_h[:], 0.0, feat2[:], AluOpType.max, AluOpType.add)
                return featn

            def emit_pool(t, feat2):
                for h in (0, 1):
                    b = 2 * t + h
                    nc.tensor.matmul(ps_pool[0:FD, 4 * b:4 * b + 4],
                                     feat2[:, FD * h:FD * h + FD], poolm_t[:],
                                     start=True, stop=True)

            # interleave IW pairs at every stage to hide dependency latency
            IW = 3
            t0 = 0
            while t0 < NPAIR:
                ts = list(range(t0, min(t0 + IW, NPAIR)))
                t0 += IW
                fts = [emit_l1(t) for t in ts]
                for l in (1, 2):
                    fts = [emit_layer(l, ts[i], fts[i]) for i in range(len(ts))]
                for i in range(len(ts)):
                    emit_pool(ts[i], fts[i])

            # --- readout MLP: relu(x @ Wr1 + br1) @ Wr2 + br2, feature-major ---
            xfm = outp.tile([FD, GPC], BF16, tag="xfm")
            nc.vector.tensor_copy(xfm[:], ps_pool[:])
            ps_t1 = pp_agg.tile([128, GPC], F32, tag="agg")
            ps_t2 = pp_agg.tile([128, GPC], F32, tag="agg")
            nc.tensor.matmul(ps_t1[:], wr1_t[:, 0:128], xfm[:], start=True, stop=True)
            nc.tensor.matmul(ps_t2[:], wr1_t[:, 128:256], xfm[:], start=True, stop=True)
            hid1 = outp.tile([128, GPC], BF16, tag="hid1")
            hid2 = outp.tile([128, GPC], BF16, tag="hid2")
            nc.vector.tensor_scalar(hid1[:], ps_t1[:], br1_t[:, 0:1], 0.0,
                                    AluOpType.add, AluOpType.max)
            nc.vector.tensor_scalar(hid2[:], ps_t2[:], br1_t[:, 1:2], 0.0,
                                    AluOpType.add, AluOpType.max)
            ps_o = pp_agg.tile([LAT, GPC], F32, tag="agg")
            nc.tensor.matmul(ps_o[:], wr2_t[:, 0:128], hid1[:], start=True, stop=False)
            nc.tensor.matmul(ps_o[:], wr2_t[:, 128:256], hid2[:], start=False, stop=True)
            out_sb = outp.tile([LAT, GPC], F32, tag="out")
            nc.vector.tensor_scalar(out_sb[:], ps_o[:], br2_t[:], None, AluOpType.add)
            nc.sync.dma_start(out_ap[:], out_sb[:])

    nc.compile()
    return nc


def kernel(pos, emb, W_s2n, W1, W2, W3, W4, Ws, Wv, Wr1, br1, Wr2, br2,
           z, batch, edge_index, num_graphs):
    pos = np.asarray(pos, dtype=np.float32)
    z = np.asarray(z)
    emb = np.asarray(emb, dtype=np.float32)
    W_s2n = np.asarray(W_s2n, dtype=np.float32)
    W1 = np.asarray(W1, dtype=np.float32); W2 = np.asarray(W2, dtype=np.float32)
    W3 = np.asarray(W3, dtype=np.float32); W4 = np.asarray(W4, dtype=np.float32)
    Ws = np.asarray(Ws, dtype=np.float32); Wv = np.asarray(Wv, dtype=np.float32)
    Wr1 = np.asarray(Wr1, dtype=np.float32); br1 = np.asarray(br1, dtype=np.float32)
    Wr2 = np.asarray(Wr2, dtype=np.float32); br2 = np.asarray(br2, dtype=np.float32)

    # host prep: embedding lookup folded with input linear
    EW = (emb @ W_s2n) * np.float32(1.0 / np.sqrt(S_MUL))     # [100, 32]
    s0 = EW[z]                                                # [N, 32]

    # masked adjacency + spherical harmonics: gm[b, src, (type, dst)]
    pos_g = pos.reshape(B, NA, 3)
    diff = pos_g[:, None, :, :] - pos_g[:, :, None, :]        # [B, s, d, c] = pos[d]-pos[s]
    d2 = (diff * diff).sum(-1)
    mask = ((d2 <= 25.0) & (d2 > 0.0)).astype(np.float32)
    with np.errstate(divide="ignore", invalid="ignore"):
        inv_r = np.float32(np.sqrt(3.0)) / np.sqrt(d2)
    inv_r[~np.isfinite(inv_r)] = 0.0
    sh = diff * (mask * inv_r)[..., None]                     # [B, s, d, 3]
    NB4 = B // GPB
    Tall = np.empty((4, NB4, GPB, NA, NA), np.float32)
    Tall[0] = mask.reshape(NB4, GPB, NA, NA)
    for c in range(3):
        Tall[1 + c] = sh[..., c].reshape(NB4, GPB, NA, NA)
    TT = np.zeros((NB4, GPB, NA, 4, GPB, NA), np.float32)
    for g in range(GPB):
        TT[:, g, :, :, g, :] = Tall[:, :, g].transpose(1, 2, 0, 3)
    gm_all = TT.reshape(NB4, 128, 512)

    # folded tensor-product + linear weights
    cs = C_SCALAR * np.float32(1.0 / np.sqrt(S_MUL))
    csb = C_SCALAR * np.float32(INV_SQRT3 / np.sqrt(S_MUL))
    cv = C_VECTOR * np.float32(INV_SQRT3 / np.sqrt(V_MUL))
    Wa = [cs * (W1[l] @ Ws[l]) for l in range(3)]     # [32, 32]
    Wb = [csb * (W4[l] @ Ws[l]) for l in range(3)]    # [16, 32]
    Wc = [cv * (W2[l] @ Wv[l]) for l in range(3)]     # [32, 16]
    Wd = [cv * (W3[l] @ Wv[l]) for l in range(3)]     # [16, 16]

    # layer-1 transform applied on host (s0 is host-prepped anyway)
    s0w = np.concatenate([s0 @ Wa[0], s0 @ Wc[0]], axis=1)    # [N, 48]

    # feature/psum row layout: s(0:32) vx(32:48) pad vy(64:80) pad vz(96:112)
    # stb rows: [s@Ax(0:32), s@Ay(32:64), s@Az(64:96), svd(96:112)]
    wta = np.zeros((FD, 2 * FD), np.float32)
    wtb = np.zeros((FD, 2 * FD), np.float32)
    for l in (1, 2):
        o = FD * (l - 1)
        wta[0:32, o:o + 32] = Wa[l]
        for c in range(3):
            r = 32 * (c + 1)
            wta[r:r + 16, o + r:o + r + 16] = Wd[l]
        for c in range(3):
            wtb[32 * c:32 * c + 32, o + 32 * (c + 1):o + 32 * (c + 1) + 16] = Wc[l]
        wtb[96:112, o:o + 32] = Wb[l]

    poolm = np.zeros((128, GPB), np.float32)
    for g in range(GPB):
        poolm[g * NA:(g + 1) * NA, g] = 1.0

    # readout weights: v rows at 32*(c+1)+u map to original 32+3u+c
    wr1p = np.zeros((FD, HID), np.float32)
    wr1p[0:32] = Wr1[0:32]
    for c in range(3):
        for u in range(V_MUL):
            wr1p[32 * (c + 1) + u] = Wr1[32 + 3 * u + c]
    wr2p = np.zeros((128, HID), np.float32)
    wr2p[:, 0:128] = Wr2[0:128]
    wr2p[:, 128:256] = Wr2[128:256]
    br1t = br1.reshape(2, 128).T.copy()               # [128, 2]
    br2t = br2.reshape(LAT, 1)

    if "nc" not in _CACHE:
        _CACHE["nc"] = _build_program()
    nc = _CACHE["nc"]

    consts = dict(
        wta=wta.astype(BF16NP), wtb=wtb.astype(BF16NP),
        poolm=poolm.astype(BF16NP), wr1=wr1p.astype(BF16NP), wr2=wr2p.astype(BF16NP),
        br1=np.ascontiguousarray(br1t), br2=br2t,
    )
    in_maps = []
    for c in range(NCORES):
        gm_core = np.ascontiguousarray(
            gm_all[c * NBLK:(c + 1) * NBLK].transpose(1, 0, 2)
        ).reshape(128, NBLK * 512).astype(BF16NP)
        s0_core = np.ascontiguousarray(
            s0[c * NPC:(c + 1) * NPC].reshape(NBLK, 128, S_MUL).transpose(1, 0, 2)
        ).reshape(128, NBLK * S_MUL).astype(BF16NP)
        s0w_core = np.ascontiguousarray(
            s0w[c * NPC:(c + 1) * NPC].reshape(NBLK, 128, 48).transpose(1, 0, 2)
        ).reshape(128, NBLK * 48).astype(BF16NP)
        m = dict(consts)
        for k in range(GCH):
            m[f"gm{k}"] = np.ascontiguousarray(
                gm_core[:, k * BPCH * 512:(k + 1) * BPCH * 512])
        m["s0"] = s0_core
        m["s0w"] = s0w_core
        in_maps.append(m)

    res = run_bass_kernel_spmd(nc, in_maps, core_ids=list(range(NCORES)))
    out = np.empty((B, LAT), np.float32)
    for c in range(NCORES):
        out[c * GPC:(c + 1) * GPC] = res.results[c]["outfm"].T
    return out


# revision 22
# speedup vs baseline: 3.2173x; 1.0487x over previous
import sys

sys.path.insert(0, "/opt/trn_rl_repo")
import numpy as np
import ml_dtypes
import concourse.bass as bass
import concourse.tile as tile
from concourse import bacc, mybir
from concourse.alu_op_type import AluOpType
from concourse.bass_utils import run_bass_kernel_spmd

# Problem constants (nn_EquivGNNEncoder: 2048 graphs x 32 atoms, 3 layers)
B, NA = 2048, 32
N = B * NA                  # 65536 nodes
S_MUL, V_MUL = 32, 16
NCORES = 8
GPC = B // NCORES           # 256 graphs per core
NPC = GPC * NA              # 8192 nodes per core
GPB = 4                     # graphs per block (4*32 = 128 partitions)
NBLK = GPC // GPB           # 64 blocks per core
NPAIR = NBLK // 2           # 32 block-pairs per core
GCH = 4                     # gm DMA chunks
BPCH = NBLK // GCH          # blocks per gm chunk
LAT = 128
HID = 256
# padded feature layout (partition ranges must start at multiples of 32):
# s(0:32) vx(32:48) pad(48:64) vy(64:80) pad(80:96) vz(96:112)
FD = 112

INV_SQRT3 = 1.0 / np.sqrt(3.0)
C_SCALAR = np.float32(1.0 / np.sqrt(48.0))
C_VECTOR = np.float32(np.sqrt(3.0 / 48.0))

F32 = mybir.dt.float32
BF16 = mybir.dt.bfloat16
BF16NP = ml_dtypes.bfloat16

_CACHE = {}


def _rf(apx, dims):
    """Return a copy of AP with the free dims replaced (partition dim kept)."""
    return bass.AP(tensor=apx.tensor, offset=apx.offset,
                   ap=[list(apx.ap[0])] + [list(d) for d in dims])


def _build_program():
    nc = bacc.Bacc("TRN2", target_bir_lowering=False, debug=False)

    gm_aps = [
        nc.dram_tensor(f"gm{k}", [128, BPCH * 512], BF16, kind="ExternalInput").ap()
        for k in range(GCH)
    ]
    s0_ap = nc.dram_tensor("s0", [128, NBLK * S_MUL], BF16, kind="ExternalInput").ap()
    s0w_ap = nc.dram_tensor("s0w", [128, NBLK * 48], BF16, kind="ExternalInput").ap()
    wta_ap = nc.dram_tensor("wta", [FD, 2 * FD], BF16, kind="ExternalInput").ap()
    wtb_ap = nc.dram_tensor("wtb", [FD, 2 * FD], BF16, kind="ExternalInput").ap()
    poolm_ap = nc.dram_tensor("poolm", [128, GPB], BF16, kind="ExternalInput").ap()
    wr1_ap = nc.dram_tensor("wr1", [FD, HID], BF16, kind="ExternalInput").ap()
    wr2_ap = nc.dram_tensor("wr2", [128, HID], BF16, kind="ExternalInput").ap()
    br1_ap = nc.dram_tensor("br1", [128, 2], F32, kind="ExternalInput").ap()
    br2_ap = nc.dram_tensor("br2", [128, 1], F32, kind="ExternalInput").ap()
    out_ap = nc.dram_tensor("outfm", [LAT, GPC], F32, kind="ExternalOutput").ap()

    with tile.TileContext(nc) as tc:
        with tc.tile_pool(name="const", bufs=1) as const, \
             tc.tile_pool(name="feat", bufs=8) as featp, \
             tc.tile_pool(name="work", bufs=4) as work, \
             tc.tile_pool(name="outp", bufs=1) as outp, \
             tc.tile_pool(name="f3", bufs=22) as f3p, \
             tc.tile_pool(name="psagg", bufs=2, space="PSUM") as pp_agg, \
             tc.tile_pool(name="psh", bufs=2, space="PSUM") as pp_h:

            # --- resident inputs + constants ---
            gm_ts = []
            for k in range(GCH):
                g = const.tile([128, BPCH * 512], BF16, tag=f"gm{k}")
                nc.sync.dma_start(g[:], gm_aps[k][:])
                gm_ts.append(g)
            s0_t = const.tile([128, NBLK * S_MUL], BF16)
            nc.sync.dma_start(s0_t[:], s0_ap[:])
            s0w_t = const.tile([128, NBLK * 48], BF16)
            nc.sync.dma_start(s0w_t[:], s0w_ap[:])
            wta_t = const.tile([FD, 2 * FD], BF16)
            nc.sync.dma_start(wta_t[:], wta_ap[:])
            wtb_t = const.tile([FD, 2 * FD], BF16)
            nc.sync.dma_start(wtb_t[:], wtb_ap[:])
            poolm_t = const.tile([128, GPB], BF16)
            nc.sync.dma_start(poolm_t[:], poolm_ap[:])
            wr1_t = const.tile([FD, HID], BF16)
            nc.sync.dma_start(wr1_t[:], wr1_ap[:])
            wr2_t = const.tile([128, HID], BF16)
            nc.sync.dma_start(wr2_t[:], wr2_ap[:])
            br1_t = const.tile([128, 2], F32)
            nc.sync.dma_start(br1_t[:], br1_ap[:])
            br2_t = const.tile([128, 1], F32)
            nc.sync.dma_start(br2_t[:], br2_ap[:])

            TB = 3                      # blocks per group
            NG = (NBLK + TB - 1) // TB  # 21 groups of 3 + 1 of 1

            def grp_blocks(g):
                b0 = TB * g
                return b0, min(TB, NBLK - b0)

            def gm_rhs(b):
                c0 = (b % BPCH) * 512
                return gm_ts[b // BPCH][:, c0:c0 + 512]

            def gm_blk(b, k):
                c0 = (b % BPCH) * 512 + 128 * k
                return gm_ts[b // BPCH][:, c0:c0 + 128]

            def g_src(ps, r0, r1, cbase, n):
                # rows r0:r1 of ps_agg, n column windows 512 apart
                return _rf(ps[r0:r1, cbase:cbase + 128], [[512, n], [1, 128]])

            def g_dst(sl, n):
                return _rf(sl, [[128, n], [1, 128]])

            def emit_l1(g):
                # transform-first layer 1: s0W precomputed on host, so each
                # output group is one adjacency^T @ s0W matmul, no copies
                b0, n = grp_blocks(g)
                ps_h = pp_h.tile([128, TB * FD], F32, tag="psh")
                for h in range(n):
                    b = b0 + h
                    sa = s0w_t[:, 48 * b:48 * b + 32]
                    sc = s0w_t[:, 48 * b + 32:48 * b + 48]
                    o = FD * h
                    nc.tensor.matmul(ps_h[:, o:o + 32], gm_blk(b, 0), sa,
                                     start=True, stop=True)
                    for k in range(3):
                        nc.tensor.matmul(
                            ps_h[:, o + 32 * (k + 1):o + 32 * (k + 1) + 16],
                            gm_blk(b, 1 + k), sc, start=True, stop=True)
                featn = featp.tile([128, TB * FD], BF16, tag="feat")
                s0grp = _rf(s0_t[:, 32 * b0:32 * (b0 + n)], [[32, n], [1, 32]])
                nc.vector.memset(
                    _rf(featn[:, 48:64], [[FD, n], [32, 2], [1, 16]]), 0.0)
                nc.vector.scalar_tensor_tensor(
                    _rf(featn[:, 0:32], [[FD, n], [1, 32]]),
                    _rf(ps_h[:, 0:32], [[FD, n], [1, 32]]),
                    0.0, s0grp, AluOpType.max, AluOpType.add)
                nc.scalar.activation(
                    _rf(featn[:, 32:48], [[FD, n], [32, 3], [1, 16]]),
                    _rf(ps_h[:, 32:48], [[FD, n], [32, 3], [1, 16]]),
                    mybir.ActivationFunctionType.Relu)
                return featn

            def emit_layer(l, g, feat2):
                b0, n = grp_blocks(g)
                ps_agg = pp_agg.tile([FD, TB * 512], F32, tag="agg")
                for h in range(n):
                    nc.tensor.matmul(
                        ps_agg[0:FD, 512 * h:512 * h + 512],
                        feat2[:, FD * h:FD * h + FD], gm_rhs(b0 + h),
                        start=True, stop=True)
                # svd chain first on Act (longest path), cA first on DVE
                tmpa = work.tile([16, TB * 128], F32, tag="tmpa")
                nc.scalar.copy(g_dst(tmpa[:, 0:128], n), g_src(ps_agg, 32, 48, 128, n))
                # ta: all features aggregated with plain adjacency A (pads = 0)
                ta = work.tile([FD, TB * 128], BF16, tag="ta")
                nc.vector.tensor_copy(g_dst(ta[:, 0:128], n), g_src(ps_agg, 0, FD, 0, n))
                # stb rows: s@Ax(0:32) s@Ay(32:64) s@Az(64:96) svd(96:112)
                stb = work.tile([FD, TB * 128], BF16, tag="stb")
                nc.scalar.copy(g_dst(stb[0:32, 0:128], n), g_src(ps_agg, 0, 32, 128, n))
                nc.scalar.copy(g_dst(stb[64:96, 0:128], n), g_src(ps_agg, 0, 32, 384, n))
                tmpb = work.tile([16, TB * 128], F32, tag="tmpb")
                nc.vector.tensor_add(g_dst(tmpb[:, 0:128], n), g_dst(tmpa[:, 0:128], n),
                                     g_src(ps_agg, 64, 80, 256, n))
                nc.vector.tensor_add(g_dst(stb[96:112, 0:128], n), g_dst(tmpb[:, 0:128], n),
                                     g_src(ps_agg, 96, 112, 384, n))
                nc.vector.tensor_copy(g_dst(stb[32:64, 0:128], n),
                                      g_src(ps_agg, 0, 32, 256, n))
                ps_h = pp_h.tile([128, TB * FD], F32, tag="psh")
                wl_a = wta_t[:, FD * (l - 1):FD * l]
                wl_b = wtb_t[:, FD * (l - 1):FD * l]
                for h in range(n):
                    nc.tensor.matmul(ps_h[:, FD * h:FD * h + FD],
                                     ta[:, 128 * h:128 * h + 128], wl_a,
                                     start=True, stop=False)
                    nc.tensor.matmul(ps_h[:, FD * h:FD * h + FD],
                                     stb[:, 128 * h:128 * h + 128], wl_b,
                                     start=False, stop=True)
                pool = f3p if l == 2 else featp
                featn = pool.tile([128, TB * FD], BF16, tag="f3" if l == 2 else "feat")
                w = FD * n
                nc.vector.scalar_tensor_tensor(
                    featn[:, 0:w], ps_h[:, 0:w], 0.0, feat2[:, 0:w],
                    AluOpType.max, AluOpType.add)
                return featn

            # interleave two groups at every stage to hide dependency latency
            f3s = [None] * NG
            for gg in range(0, NG, 2):
                gs = [g for g in (gg, gg + 1) if g < NG]
                fts = [emit_l1(g) for g in gs]
                for l in (1, 2):
                    fts = [emit_layer(l, gs[i], fts[i]) for i in range(len(gs))]
                for i in range(len(gs)):
                    f3s[gs[i]] = fts[i]

            # sum-pool all graphs from the retained layer-3 features
            ps_pool = pp_agg.tile([FD, GPC], F32, tag="agg")
            for g in range(NG):
                b0, n = grp_blocks(g)
                for h in range(n):
                    b = b0 + h
                    nc.tensor.matmul(ps_pool[0:FD, 4 * b:4 * b + 4],
                                     f3s[g][:, FD * h:FD * h + FD], poolm_t[:],
                                     start=True, stop=True)

            # --- readout MLP: relu(x @ Wr1 + br1) @ Wr2 + br2, feature-major ---
            xfm = outp.tile([FD, GPC], BF16, tag="xfm")
            nc.vector.tensor_copy(xfm[:], ps_pool[:])
            ps_t1 = pp_agg.tile([128, GPC], F32, tag="agg")
            ps_t2 = pp_agg.tile([128, GPC], F32, tag="agg")
            nc.tensor.matmul(ps_t1[:], wr1_t[:, 0:128], xfm[:], start=True, stop=True)
            nc.tensor.matmul(ps_t2[:], wr1_t[:, 128:256], xfm[:], start=True, stop=True)
            hid1 = outp.tile([128, GPC], BF16, tag="hid1")
            hid2 = outp.tile([128, GPC], BF16, tag="hid2")
            nc.vector.tensor_scalar(hid1[:], ps_t1[:], br1_t[:, 0:1], 0.0,
                                    AluOpType.add, AluOpType.max)
            nc.vector.tensor_scalar(hid2[:], ps_t2[:], br1_t[:, 1:2], 0.0,
                                    AluOpType.add, AluOpType.max)
            ps_o = pp_agg.tile([LAT, GPC], F32, tag="agg")
            nc.tensor.matmul(ps_o[:], wr2_t[:, 0:128], hid1[:], start=True, stop=False)
            nc.tensor.matmul(ps_o[:], wr2_t[:, 128:256], hid2[:], start=False, stop=True)
            out_sb = outp.tile([LAT, GPC], F32, tag="out")
            nc.vector.tensor_scalar(out_sb[:], ps_o[:], br2_t[:], None, AluOpType.add)
            nc.sync.dma_start(out_ap[:], out_sb[:])

    nc.compile()
    return nc


def kernel(pos, emb, W_s2n, W1, W2, W3, W4, Ws, Wv, Wr1, br1, Wr2, br2,
           z, batch, edge_index, num_graphs):
    pos = np.asarray(pos, dtype=np.float32)
    z = np.asarray(z)
    emb = np.asarray(emb, dtype=np.float32)
    W_s2n = np.asarray(W_s2n, dtype=np.float32)
    W1 = np.asarray(W1, dtype=np.float32); W2 = np.asarray(W2, dtype=np.float32)
    W3 = np.asarray(W3, dtype=np.float32); W4 = np.asarray(W4, dtype=np.float32)
    Ws = np.asarray(Ws, dtype=np.float32); Wv = np.asarray(Wv, dtype=np.float32)
    Wr1 = np.asarray(Wr1, dtype=np.float32); br1 = np.asarray(br1, dtype=np.float32)
    Wr2 = np.asarray(Wr2, dtype=np.float32); br2 = np.asarray(br2, dtype=np.float32)

    # host prep: embedding lookup folded with input linear
    EW = (emb @ W_s2n) * np.float32(1.0 / np.sqrt(S_MUL))     # [100, 32]
    s0 = EW[z]                                                # [N, 32]

    # masked adjacency + spherical harmonics: gm[b, src, (type, dst)]
    pos_g = pos.reshape(B, NA, 3)
    diff = pos_g[:, None, :, :] - pos_g[:, :, None, :]        # [B, s, d, c] = pos[d]-pos[s]
    d2 = (diff * diff).sum(-1)
    mask = ((d2 <= 25.0) & (d2 > 0.0)).astype(np.float32)
    with np.errstate(divide="ignore", invalid="ignore"):
        inv_r = np.float32(np.sqrt(3.0)) / np.sqrt(d2)
    inv_r[~np.isfinite(inv_r)] = 0.0
    sh = diff * (mask * inv_r)[..., None]                     # [B, s, d, 3]
    NB4 = B // GPB
    Tall = np.empty((4, NB4, GPB, NA, NA), np.float32)
    Tall[0] = mask.reshape(NB4, GPB, NA, NA)
    for c in range(3):
        Tall[1 + c] = sh[..., c].reshape(NB4, GPB, NA, NA)
    TT = np.zeros((NB4, GPB, NA, 4, GPB, NA), np.float32)
    for g in range(GPB):
        TT[:, g, :, :, g, :] = Tall[:, :, g].transpose(1, 2, 0, 3)
    gm_all = TT.reshape(NB4, 128, 512)

    # folded tensor-product + linear weights
    cs = C_SCALAR * np.float32(1.0 / np.sqrt(S_MUL))
    csb = C_SCALAR * np.float32(INV_SQRT3 / np.sqrt(S_MUL))
    cv = C_VECTOR * np.float32(INV_SQRT3 / np.sqrt(V_MUL))
    Wa = [cs * (W1[l] @ Ws[l]) for l in range(3)]     # [32, 32]
    Wb = [csb * (W4[l] @ Ws[l]) for l in range(3)]    # [16, 32]
    Wc = [cv * (W2[l] @ Wv[l]) for l in range(3)]     # [32, 16]
    Wd = [cv * (W3[l] @ Wv[l]) for l in range(3)]     # [16, 16]

    # layer-1 transform applied on host (s0 is host-prepped anyway)
    s0w = np.concatenate([s0 @ Wa[0], s0 @ Wc[0]], axis=1)    # [N, 48]

    # feature/psum row layout: s(0:32) vx(32:48) pad vy(64:80) pad vz(96:112)
    # stb rows: [s@Ax(0:32), s@Ay(32:64), s@Az(64:96), svd(96:112)]
    wta = np.zeros((FD, 2 * FD), np.float32)
    wtb = np.zeros((FD, 2 * FD), np.float32)
    for l in (1, 2):
        o = FD * (l - 1)
        wta[0:32, o:o + 32] = Wa[l]
        for c in range(3):
            r = 32 * (c + 1)
            wta[r:r + 16, o + r:o + r + 16] = Wd[l]
        for c in range(3):
            wtb[32 * c:32 * c + 32, o + 32 * (c + 1):o + 32 * (c + 1) + 16] = Wc[l]
        wtb[96:112, o:o + 32] = Wb[l]

    poolm = np.zeros((128, GPB), np.float32)
    for g in range(GPB):
        poolm[g * NA:(g + 1) * NA, g] = 1.0

    # readout weights: v rows at 32*(c+1)+u map to original 32+3u+c
    wr1p = np.zeros((FD, HID), np.float32)
    wr1p[0:32] = Wr1[0:32]
    for c in range(3):
        for u in range(V_MUL):
            wr1p[32 * (c + 1) + u] = Wr1[32 + 3 * u + c]
    wr2p = np.zeros((128, HID), np.float32)
    wr2p[:, 0:128] = Wr2[0:128]
    wr2p[:, 128:256] = Wr2[128:256]
    br1t = br1.reshape(2, 128).T.copy()               # [128, 2]
    br2t = br2.reshape(LAT, 1)

    if "nc" not in _CACHE:
        _CACHE["nc"] = _build_program()
    nc = _CACHE["nc"]

    consts = dict(
        wta=wta.astype(BF16NP), wtb=wtb.astype(BF16NP),
        poolm=poolm.astype(BF16NP), wr1=wr1p.astype(BF16NP), wr2=wr2p.astype(BF16NP),
        br1=np.ascontiguousarray(br1t), br2=br2t,
    )
    in_maps = []
    for c in range(NCORES):
        gm_core = np.ascontiguousarray(
            gm_all[c * NBLK:(c + 1) * NBLK].transpose(1, 0, 2)
        ).reshape(128, NBLK * 512).astype(BF16NP)
        s0_core = np.ascontiguousarray(
            s0[c * NPC:(c + 1) * NPC].reshape(NBLK, 128, S_MUL).transpose(1, 0, 2)
        ).reshape(128, NBLK * S_MUL).astype(BF16NP)
        s0w_core = np.ascontiguousarray(
            s0w[c * NPC:(c + 1) * NPC].reshape(NBLK, 128, 48).transpose(1, 0, 2)
        ).reshape(128, NBLK * 48).astype(BF16NP)
        m = dict(consts)
        for k in range(GCH):
            m[f"gm{k}"] = np.ascontiguousarray(
                gm_core[:, k * BPCH * 512:(k + 1) * BPCH * 512])
        m["s0"] = s0_core
        m["s0w"] = s0w_core
        in_maps.append(m)

    res = run_bass_kernel_spmd(nc, in_maps, core_ids=list(range(NCORES)))
    out = np.empty((B, LAT), np.float32)
    for c in range(NCORES):
        out[c * GPC:(c + 1) * GPC] = res.results[c]["outfm"].T
    return out


# revision 23
# speedup vs baseline: 3.6398x; 1.1313x over previous
import sys

sys.path.insert(0, "/opt/trn_rl_repo")
import numpy as np
import ml_dtypes
import concourse.bass as bass
import concourse.tile as tile
from concourse import bacc, mybir
from concourse.alu_op_type import AluOpType
from concourse.bass_utils import run_bass_kernel_spmd

# Problem constants (nn_EquivGNNEncoder: 2048 graphs x 32 atoms, 3 layers)
B, NA = 2048, 32
N = B * NA                  # 65536 nodes
S_MUL, V_MUL = 32, 16
NCORES = 8
GPC = B // NCORES           # 256 graphs per core
NPC = GPC * NA              # 8192 nodes per core
GPB = 4                     # graphs per block (4*32 = 128 partitions)
NBLK = GPC // GPB           # 64 blocks per core
NPAIR = NBLK // 2           # 32 block-pairs per core
GCH = 4                     # gm DMA chunks
BPCH = NBLK // GCH          # blocks per gm chunk
LAT = 128
HID = 256
# padded feature layout (partition ranges must start at multiples of 32):
# s(0:32) vx(32:48) pad(48:64) vy(64:80) pad(80:96) vz(96:112)
FD = 112

INV_SQRT3 = 1.0 / np.sqrt(3.0)
C_SCALAR = np.float32(1.0 / np.sqrt(48.0))
C_VECTOR = np.float32(np.sqrt(3.0 / 48.0))

F32 = mybir.dt.float32
BF16 = mybir.dt.bfloat16
BF16NP = ml_dtypes.bfloat16

_CACHE = {}


def _rf(apx, dims):
    """Return a copy of AP with the free dims replaced (partition dim kept)."""
    return bass.AP(tensor=apx.tensor, offset=apx.offset,
                   ap=[list(apx.ap[0])] + [list(d) for d in dims])


def _build_program():
    nc = bacc.Bacc("TRN2", target_bir_lowering=False, debug=False)

    gm_aps = [
        nc.dram_tensor(f"gm{k}", [128, BPCH * 512], BF16, kind="ExternalInput").ap()
        for k in range(GCH)
    ]
    s0_ap = nc.dram_tensor("s0", [128, NBLK * S_MUL], BF16, kind="ExternalInput").ap()
    s0w_ap = nc.dram_tensor("s0w", [128, NBLK * 48], BF16, kind="ExternalInput").ap()
    wta_ap = nc.dram_tensor("wta", [FD, 2 * FD], BF16, kind="ExternalInput").ap()
    wtb_ap = nc.dram_tensor("wtb", [FD, 2 * FD], BF16, kind="ExternalInput").ap()
    poolm_ap = nc.dram_tensor("poolm", [128, GPB], BF16, kind="ExternalInput").ap()
    wr1_ap = nc.dram_tensor("wr1", [FD, HID], BF16, kind="ExternalInput").ap()
    wr2_ap = nc.dram_tensor("wr2", [128, HID], BF16, kind="ExternalInput").ap()
    br1_ap = nc.dram_tensor("br1", [128, 2], F32, kind="ExternalInput").ap()
    br2_ap = nc.dram_tensor("br2", [128, 1], F32, kind="ExternalInput").ap()
    out_ap = nc.dram_tensor("outfm", [LAT, GPC], F32, kind="ExternalOutput").ap()

    with tile.TileContext(nc) as tc:
        with tc.tile_pool(name="const", bufs=1) as const, \
             tc.tile_pool(name="work", bufs=4) as work, \
             tc.tile_pool(name="outp", bufs=1) as outp, \
             tc.tile_pool(name="f1", bufs=23) as f1p, \
             tc.tile_pool(name="f2", bufs=23) as f2p, \
             tc.tile_pool(name="f3", bufs=23) as f3p, \
             tc.tile_pool(name="psagg", bufs=2, space="PSUM") as pp_agg, \
             tc.tile_pool(name="psh", bufs=2, space="PSUM") as pp_h:

            # --- resident inputs + constants ---
            gm_ts = []
            for k in range(GCH):
                g = const.tile([128, BPCH * 512], BF16, tag=f"gm{k}")
                nc.sync.dma_start(g[:], gm_aps[k][:])
                gm_ts.append(g)
            s0_t = const.tile([128, NBLK * S_MUL], BF16)
            nc.sync.dma_start(s0_t[:], s0_ap[:])
            s0w_t = const.tile([128, NBLK * 48], BF16)
            nc.sync.dma_start(s0w_t[:], s0w_ap[:])
            wta_t = const.tile([FD, 2 * FD], BF16)
            nc.sync.dma_start(wta_t[:], wta_ap[:])
            wtb_t = const.tile([FD, 2 * FD], BF16)
            nc.sync.dma_start(wtb_t[:], wtb_ap[:])
            poolm_t = const.tile([128, GPB], BF16)
            nc.sync.dma_start(poolm_t[:], poolm_ap[:])
            wr1_t = const.tile([FD, HID], BF16)
            nc.sync.dma_start(wr1_t[:], wr1_ap[:])
            wr2_t = const.tile([128, HID], BF16)
            nc.sync.dma_start(wr2_t[:], wr2_ap[:])
            br1_t = const.tile([128, 2], F32)
            nc.sync.dma_start(br1_t[:], br1_ap[:])
            br2_t = const.tile([128, 1], F32)
            nc.sync.dma_start(br2_t[:], br2_ap[:])

            TB = 3                      # blocks per group
            NG = (NBLK + TB - 1) // TB  # 21 groups of 3 + 1 of 1

            def grp_blocks(g):
                b0 = TB * g
                return b0, min(TB, NBLK - b0)

            def gm_rhs(b):
                c0 = (b % BPCH) * 512
                return gm_ts[b // BPCH][:, c0:c0 + 512]

            def gm_blk(b, k):
                c0 = (b % BPCH) * 512 + 128 * k
                return gm_ts[b // BPCH][:, c0:c0 + 128]

            def g_src(ps, r0, r1, cbase, n):
                # rows r0:r1 of ps_agg, n column windows 512 apart
                return _rf(ps[r0:r1, cbase:cbase + 128], [[512, n], [1, 128]])

            def g_dst(sl, n):
                return _rf(sl, [[128, n], [1, 128]])

            def emit_l1(g):
                # transform-first layer 1: s0W precomputed on host, so each
                # output group is one adjacency^T @ s0W matmul, no copies
                b0, n = grp_blocks(g)
                ps_h = pp_h.tile([128, TB * FD], F32, tag="psh")
                for h in range(n):
                    b = b0 + h
                    sa = s0w_t[:, 48 * b:48 * b + 32]
                    sc = s0w_t[:, 48 * b + 32:48 * b + 48]
                    o = FD * h
                    nc.tensor.matmul(ps_h[:, o:o + 32], gm_blk(b, 0), sa,
                                     start=True, stop=True)
                    for k in range(3):
                        nc.tensor.matmul(
                            ps_h[:, o + 32 * (k + 1):o + 32 * (k + 1) + 16],
                            gm_blk(b, 1 + k), sc, start=True, stop=True)
                featn = f1p.tile([128, TB * FD], BF16, tag="f1")
                s0grp = _rf(s0_t[:, 32 * b0:32 * (b0 + n)], [[32, n], [1, 32]])
                nc.vector.memset(
                    _rf(featn[:, 48:64], [[FD, n], [32, 2], [1, 16]]), 0.0)
                nc.vector.scalar_tensor_tensor(
                    _rf(featn[:, 0:32], [[FD, n], [1, 32]]),
                    _rf(ps_h[:, 0:32], [[FD, n], [1, 32]]),
                    0.0, s0grp, AluOpType.max, AluOpType.add)
                nc.scalar.activation(
                    _rf(featn[:, 32:48], [[FD, n], [32, 3], [1, 16]]),
                    _rf(ps_h[:, 32:48], [[FD, n], [32, 3], [1, 16]]),
                    mybir.ActivationFunctionType.Relu)
                return featn

            def emit_layer(l, g, feat2):
                b0, n = grp_blocks(g)
                ps_agg = pp_agg.tile([FD, TB * 512], F32, tag="agg")
                for h in range(n):
                    nc.tensor.matmul(
                        ps_agg[0:FD, 512 * h:512 * h + 512],
                        feat2[:, FD * h:FD * h + FD], gm_rhs(b0 + h),
                        start=True, stop=True)
                # svd chain head first on Act (it gates the V adds)
                tmpa = work.tile([16, TB * 128], F32, tag="tmpa")
                nc.scalar.copy(g_dst(tmpa[:, 0:128], n), g_src(ps_agg, 32, 48, 128, n))
                # ta: all features aggregated with plain adjacency A (pads = 0)
                ta = work.tile([FD, TB * 128], BF16, tag="ta")
                nc.scalar.copy(g_dst(ta[:, 0:128], n), g_src(ps_agg, 0, FD, 0, n))
                # stb rows: s@Ax(0:32) s@Ay(32:64) s@Az(64:96) svd(96:112)
                stb = work.tile([FD, TB * 128], BF16, tag="stb")
                nc.vector.tensor_copy(g_dst(stb[32:64, 0:128], n),
                                      g_src(ps_agg, 0, 32, 256, n))
                nc.scalar.copy(g_dst(stb[0:32, 0:128], n), g_src(ps_agg, 0, 32, 128, n))
                nc.scalar.copy(g_dst(stb[64:96, 0:128], n), g_src(ps_agg, 0, 32, 384, n))
                tmpb = work.tile([16, TB * 128], F32, tag="tmpb")
                nc.vector.tensor_add(g_dst(tmpb[:, 0:128], n), g_dst(tmpa[:, 0:128], n),
                                     g_src(ps_agg, 64, 80, 256, n))
                nc.vector.tensor_add(g_dst(stb[96:112, 0:128], n), g_dst(tmpb[:, 0:128], n),
                                     g_src(ps_agg, 96, 112, 384, n))
                ps_h = pp_h.tile([128, TB * FD], F32, tag="psh")
                wl_a = wta_t[:, FD * (l - 1):FD * l]
                wl_b = wtb_t[:, FD * (l - 1):FD * l]
                for h in range(n):
                    nc.tensor.matmul(ps_h[:, FD * h:FD * h + FD],
                                     ta[:, 128 * h:128 * h + 128], wl_a,
                                     start=True, stop=False)
                    nc.tensor.matmul(ps_h[:, FD * h:FD * h + FD],
                                     stb[:, 128 * h:128 * h + 128], wl_b,
                                     start=False, stop=True)
                pool = f3p if l == 2 else f2p
                featn = pool.tile([128, TB * FD], BF16, tag="f3" if l == 2 else "f2")
                w = FD * n
                nc.vector.scalar_tensor_tensor(
                    featn[:, 0:w], ps_h[:, 0:w], 0.0, feat2[:, 0:w],
                    AluOpType.max, AluOpType.add)
                return featn

            # phase-sequential: every group through L1, then L2, then L3 —
            # each phase is NG independent chains so engines stay saturated
            f1s = [emit_l1(g) for g in range(NG)]
            f2s = [emit_layer(1, g, f1s[g]) for g in range(NG)]
            f3s = [emit_layer(2, g, f2s[g]) for g in range(NG)]

            # sum-pool all graphs from the retained layer-3 features
            ps_pool = pp_agg.tile([FD, GPC], F32, tag="agg")
            for g in range(NG):
                b0, n = grp_blocks(g)
                for h in range(n):
                    b = b0 + h
                    nc.tensor.matmul(ps_pool[0:FD, 4 * b:4 * b + 4],
                                     f3s[g][:, FD * h:FD * h + FD], poolm_t[:],
                                     start=True, stop=True)

            # --- readout MLP: relu(x @ Wr1 + br1) @ Wr2 + br2, feature-major ---
            xfm = outp.tile([FD, GPC], BF16, tag="xfm")
            nc.vector.tensor_copy(xfm[:], ps_pool[:])
            ps_t1 = pp_agg.tile([128, GPC], F32, tag="agg")
            ps_t2 = pp_agg.tile([128, GPC], F32, tag="agg")
            nc.tensor.matmul(ps_t1[:], wr1_t[:, 0:128], xfm[:], start=True, stop=True)
            nc.tensor.matmul(ps_t2[:], wr1_t[:, 128:256], xfm[:], start=True, stop=True)
            hid1 = outp.tile([128, GPC], BF16, tag="hid1")
            hid2 = outp.tile([128, GPC], BF16, tag="hid2")
            nc.vector.tensor_scalar(hid1[:], ps_t1[:], br1_t[:, 0:1], 0.0,
                                    AluOpType.add, AluOpType.max)
            nc.vector.tensor_scalar(hid2[:], ps_t2[:], br1_t[:, 1:2], 0.0,
                                    AluOpType.add, AluOpType.max)
            ps_o = pp_agg.tile([LAT, GPC], F32, tag="agg")
            nc.tensor.matmul(ps_o[:], wr2_t[:, 0:128], hid1[:], start=True, stop=False)
            nc.tensor.matmul(ps_o[:], wr2_t[:, 128:256], hid2[:], start=False, stop=True)
            out_sb = outp.tile([LAT, GPC], F32, tag="out")
            nc.vector.tensor_scalar(out_sb[:], ps_o[:], br2_t[:], None, AluOpType.add)
            nc.sync.dma_start(out_ap[:], out_sb[:])

    nc.compile()
    return nc


def kernel(pos, emb, W_s2n, W1, W2, W3, W4, Ws, Wv, Wr1, br1, Wr2, br2,
           z, batch, edge_index, num_graphs):
    pos = np.asarray(pos, dtype=np.float32)
    z = np.asarray(z)
    emb = np.asarray(emb, dtype=np.float32)
    W_s2n = np.asarray(W_s2n, dtype=np.float32)
    W1 = np.asarray(W1, dtype=np.float32); W2 = np.asarray(W2, dtype=np.float32)
    W3 = np.asarray(W3, dtype=np.float32); W4 = np.asarray(W4, dtype=np.float32)
    Ws = np.asarray(Ws, dtype=np.float32); Wv = np.asarray(Wv, dtype=np.float32)
    Wr1 = np.asarray(Wr1, dtype=np.float32); br1 = np.asarray(br1, dtype=np.float32)
    Wr2 = np.asarray(Wr2, dtype=np.float32); br2 = np.asarray(br2, dtype=np.float32)

    # host prep: embedding lookup folded with input linear
    EW = (emb @ W_s2n) * np.float32(1.0 / np.sqrt(S_MUL))     # [100, 32]
    s0 = EW[z]                                                # [N, 32]

    # masked adjacency + spherical harmonics: gm[b, src, (type, dst)]
    pos_g = pos.reshape(B, NA, 3)
    diff = pos_g[:, None, :, :] - pos_g[:, :, None, :]        # [B, s, d, c] = pos[d]-pos[s]
    d2 = (diff * diff).sum(-1)
    mask = ((d2 <= 25.0) & (d2 > 0.0)).astype(np.float32)
    with np.errstate(divide="ignore", invalid="ignore"):
        inv_r = np.float32(np.sqrt(3.0)) / np.sqrt(d2)
    inv_r[~np.isfinite(inv_r)] = 0.0
    sh = diff * (mask * inv_r)[..., None]                     # [B, s, d, 3]
    NB4 = B // GPB
    Tall = np.empty((4, NB4, GPB, NA, NA), np.float32)
    Tall[0] = mask.reshape(NB4, GPB, NA, NA)
    for c in range(3):
        Tall[1 + c] = sh[..., c].reshape(NB4, GPB, NA, NA)
    TT = np.zeros((NB4, GPB, NA, 4, GPB, NA), np.float32)
    for g in range(GPB):
        TT[:, g, :, :, g, :] = Tall[:, :, g].transpose(1, 2, 0, 3)
    gm_all = TT.reshape(NB4, 128, 512)

    # folded tensor-product + linear weights
    cs = C_SCALAR * np.float32(1.0 / np.sqrt(S_MUL))
    csb = C_SCALAR * np.float32(INV_SQRT3 / np.sqrt(S_MUL))
    cv = C_VECTOR * np.float32(INV_SQRT3 / np.sqrt(V_MUL))
    Wa = [cs * (W1[l] @ Ws[l]) for l in range(3)]     # [32, 32]
    Wb = [csb * (W4[l] @ Ws[l]) for l in range(3)]    # [16, 32]
    Wc = [cv * (W2[l] @ Wv[l]) for l in range(3)]     # [32, 16]
    Wd = [cv * (W3[l] @ Wv[l]) for l in range(3)]     # [16, 16]

    # layer-1 transform applied on host (s0 is host-prepped anyway)
    s0w = np.concatenate([s0 @ Wa[0], s0 @ Wc[0]], axis=1)    # [N, 48]

    # feature/psum row layout: s(0:32) vx(32:48) pad vy(64:80) pad vz(96:112)
    # stb rows: [s@Ax(0:32), s@Ay(32:64), s@Az(64:96), svd(96:112)]
    wta = np.zeros((FD, 2 * FD), np.float32)
    wtb = np.zeros((FD, 2 * FD), np.float32)
    for l in (1, 2):
        o = FD * (l - 1)
        wta[0:32, o:o + 32] = Wa[l]
        for c in range(3):
            r = 32 * (c + 1)
            wta[r:r + 16, o + r:o + r + 16] = Wd[l]
        for c in range(3):
            wtb[32 * c:32 * c + 32, o + 32 * (c + 1):o + 32 * (c + 1) + 16] = Wc[l]
        wtb[96:112, o:o + 32] = Wb[l]

    poolm = np.zeros((128, GPB), np.float32)
    for g in range(GPB):
        poolm[g * NA:(g + 1) * NA, g] = 1.0

    # readout weights: v rows at 32*(c+1)+u map to original 32+3u+c
    wr1p = np.zeros((FD, HID), np.float32)
    wr1p[0:32] = Wr1[0:32]
    for c in range(3):
        for u in range(V_MUL):
            wr1p[32 * (c + 1) + u] = Wr1[32 + 3 * u + c]
    wr2p = np.zeros((128, HID), np.float32)
    wr2p[:, 0:128] = Wr2[0:128]
    wr2p[:, 128:256] = Wr2[128:256]
    br1t = br1.reshape(2, 128).T.copy()               # [128, 2]
    br2t = br2.reshape(LAT, 1)

    if "nc" not in _CACHE:
        _CACHE["nc"] = _build_program()
    nc = _CACHE["nc"]

    consts = dict(
        wta=wta.astype(BF16NP), wtb=wtb.astype(BF16NP),
        poolm=poolm.astype(BF16NP), wr1=wr1p.astype(BF16NP), wr2=wr2p.astype(BF16NP),
        br1=np.ascontiguousarray(br1t), br2=br2t,
    )
    in_maps = []
    for c in range(NCORES):
        gm_core = np.ascontiguousarray(
            gm_all[c * NBLK:(c + 1) * NBLK].transpose(1, 0, 2)
        ).reshape(128, NBLK * 512).astype(BF16NP)
        s0_core = np.ascontiguousarray(
            s0[c * NPC:(c + 1) * NPC].reshape(NBLK, 128, S_MUL).transpose(1, 0, 2)
        ).reshape(128, NBLK * S_MUL).astype(BF16NP)
        s0w_core = np.ascontiguousarray(
            s0w[c * NPC:(c + 1) * NPC].reshape(NBLK, 128, 48).transpose(1, 0, 2)
        ).reshape(128, NBLK * 48).astype(BF16NP)
        m = dict(consts)
        for k in range(GCH):
            m[f"gm{k}"] = np.ascontiguousarray(
                gm_core[:, k * BPCH * 512:(k + 1) * BPCH * 512])
        m["s0"] = s0_core
        m["s0w"] = s0w_core
        in_maps.append(m)

    res = run_bass_kernel_spmd(nc, in_maps, core_ids=list(range(NCORES)))
    out = np.empty((B, LAT), np.float32)
    for c in range(NCORES):
        out[c * GPC:(c + 1) * GPC] = res.results[c]["outfm"].T
    return out


# revision 24
# speedup vs baseline: 4.9651x; 1.3641x over previous
import sys

sys.path.insert(0, "/opt/trn_rl_repo")
import numpy as np
import ml_dtypes
import concourse.bass as bass
import concourse.tile as tile
from concourse import bacc, mybir
from concourse.alu_op_type import AluOpType
from concourse.bass_utils import run_bass_kernel_spmd

# Problem constants (nn_EquivGNNEncoder: 2048 graphs x 32 atoms, 3 layers)
B, NA = 2048, 32
N = B * NA                  # 65536 nodes
S_MUL, V_MUL = 32, 16
NCORES = 8
GPC = B // NCORES           # 256 graphs per core
NPC = GPC * NA              # 8192 nodes per core
GPB = 4                     # graphs per block (4*32 = 128 partitions)
NBLK = GPC // GPB           # 64 blocks per core
NPAIR = NBLK // 2           # 32 block-pairs per core
GCH = 4                     # gm DMA chunks
BPCH = NBLK // GCH          # blocks per gm chunk
LAT = 128
HID = 256
# padded feature layout (partition ranges must start at multiples of 32):
# s(0:32) vx(32:48) pad(48:64) vy(64:80) pad(80:96) vz(96:112)
FD = 112

INV_SQRT3 = 1.0 / np.sqrt(3.0)
C_SCALAR = np.float32(1.0 / np.sqrt(48.0))
C_VECTOR = np.float32(np.sqrt(3.0 / 48.0))

F32 = mybir.dt.float32
BF16 = mybir.dt.bfloat16
BF16NP = ml_dtypes.bfloat16

_CACHE = {}


def _rf(apx, dims):
    """Return a copy of AP with the free dims replaced (partition dim kept)."""
    return bass.AP(tensor=apx.tensor, offset=apx.offset,
                   ap=[list(apx.ap[0])] + [list(d) for d in dims])


def _build_program():
    nc = bacc.Bacc("TRN2", target_bir_lowering=False, debug=False)

    gm_aps = [
        nc.dram_tensor(f"gm{k}", [128, BPCH * 512], BF16, kind="ExternalInput").ap()
        for k in range(GCH)
    ]
    s0_ap = nc.dram_tensor("s0", [128, NBLK * S_MUL], BF16, kind="ExternalInput").ap()
    s0w_ap = nc.dram_tensor("s0w", [128, NBLK * 48], BF16, kind="ExternalInput").ap()
    wta_ap = nc.dram_tensor("wta", [FD, 2 * FD], BF16, kind="ExternalInput").ap()
    wtb_ap = nc.dram_tensor("wtb", [FD, 2 * FD], BF16, kind="ExternalInput").ap()
    poolm_ap = nc.dram_tensor("poolm", [128, GPB], BF16, kind="ExternalInput").ap()
    wr1_ap = nc.dram_tensor("wr1", [FD, HID], BF16, kind="ExternalInput").ap()
    wr2_ap = nc.dram_tensor("wr2", [128, HID], BF16, kind="ExternalInput").ap()
    br1_ap = nc.dram_tensor("br1", [128, 2], F32, kind="ExternalInput").ap()
    br2_ap = nc.dram_tensor("br2", [128, 1], F32, kind="ExternalInput").ap()
    out_ap = nc.dram_tensor("outfm", [LAT, GPC], F32, kind="ExternalOutput").ap()

    with tile.TileContext(nc) as tc:
        with tc.tile_pool(name="const", bufs=1) as const, \
             tc.tile_pool(name="work", bufs=4) as work, \
             tc.tile_pool(name="outp", bufs=1) as outp, \
             tc.tile_pool(name="f1", bufs=23) as f1p, \
             tc.tile_pool(name="f2", bufs=23) as f2p, \
             tc.tile_pool(name="f3", bufs=23) as f3p, \
             tc.tile_pool(name="psagg", bufs=2, space="PSUM") as pp_agg, \
             tc.tile_pool(name="psh", bufs=2, space="PSUM") as pp_h:

            # --- resident inputs + constants ---
            gm_ts = []
            for k in range(GCH):
                g = const.tile([128, BPCH * 512], BF16, tag=f"gm{k}")
                nc.sync.dma_start(g[:], gm_aps[k][:])
                gm_ts.append(g)
            s0_t = const.tile([128, NBLK * S_MUL], BF16)
            nc.sync.dma_start(s0_t[:], s0_ap[:])
            s0w_t = const.tile([128, NBLK * 48], BF16)
            nc.sync.dma_start(s0w_t[:], s0w_ap[:])
            wta_t = const.tile([FD, 2 * FD], BF16)
            nc.sync.dma_start(wta_t[:], wta_ap[:])
            wtb_t = const.tile([FD, 2 * FD], BF16)
            nc.sync.dma_start(wtb_t[:], wtb_ap[:])
            poolm_t = const.tile([128, GPB], BF16)
            nc.sync.dma_start(poolm_t[:], poolm_ap[:])
            wr1_t = const.tile([FD, HID], BF16)
            nc.sync.dma_start(wr1_t[:], wr1_ap[:])
            wr2_t = const.tile([128, HID], BF16)
            nc.sync.dma_start(wr2_t[:], wr2_ap[:])
            br1_t = const.tile([128, 2], F32)
            nc.sync.dma_start(br1_t[:], br1_ap[:])
            br2_t = const.tile([128, 1], F32)
            nc.sync.dma_start(br2_t[:], br2_ap[:])

            TB = 3                      # blocks per group
            NG = (NBLK + TB - 1) // TB  # 21 groups of 3 + 1 of 1

            def grp_blocks(g):
                b0 = TB * g
                return b0, min(TB, NBLK - b0)

            def gm_rhs(b):
                c0 = (b % BPCH) * 512
                return gm_ts[b // BPCH][:, c0:c0 + 512]

            def gm_blk(b, k):
                c0 = (b % BPCH) * 512 + 128 * k
                return gm_ts[b // BPCH][:, c0:c0 + 128]

            def g_src(ps, r0, r1, cbase, n):
                # rows r0:r1 of ps_agg, n column windows 512 apart
                return _rf(ps[r0:r1, cbase:cbase + 128], [[512, n], [1, 128]])

            def g_dst(sl, n):
                return _rf(sl, [[128, n], [1, 128]])

            def emit_l1(g):
                # transform-first layer 1: s0W precomputed on host, so each
                # output group is one adjacency^T @ s0W matmul, no copies
                b0, n = grp_blocks(g)
                ps_h = pp_h.tile([128, TB * FD], F32, tag="psh")
                for h in range(n):
                    b = b0 + h
                    sa = s0w_t[:, 48 * b:48 * b + 32]
                    sc = s0w_t[:, 48 * b + 32:48 * b + 48]
                    o = FD * h
                    nc.tensor.matmul(ps_h[:, o:o + 32], gm_blk(b, 0), sa,
                                     start=True, stop=True)
                    for k in range(3):
                        nc.tensor.matmul(
                            ps_h[:, o + 32 * (k + 1):o + 32 * (k + 1) + 16],
                            gm_blk(b, 1 + k), sc, start=True, stop=True)
                featn = f1p.tile([128, TB * FD], BF16, tag="f1")
                s0grp = _rf(s0_t[:, 32 * b0:32 * (b0 + n)], [[32, n], [1, 32]])
                nc.vector.memset(
                    _rf(featn[:, 48:64], [[FD, n], [32, 2], [1, 16]]), 0.0)
                nc.vector.scalar_tensor_tensor(
                    _rf(featn[:, 0:32], [[FD, n], [1, 32]]),
                    _rf(ps_h[:, 0:32], [[FD, n], [1, 32]]),
                    0.0, s0grp, AluOpType.max, AluOpType.add)
                nc.scalar.activation(
                    _rf(featn[:, 32:48], [[FD, n], [32, 3], [1, 16]]),
                    _rf(ps_h[:, 32:48], [[FD, n], [32, 3], [1, 16]]),
                    mybir.ActivationFunctionType.Relu)
                return featn

            def emit_layer(l, g, feat2):
                b0, n = grp_blocks(g)
                ps_agg = pp_agg.tile([FD, TB * 512], F32, tag="agg")
                for h in range(n):
                    nc.tensor.matmul(
                        ps_agg[0:FD, 512 * h:512 * h + 512],
                        feat2[:, FD * h:FD * h + FD], gm_rhs(b0 + h),
                        start=True, stop=True)
                # fold the sh.v dot into PSUM: vy@gm_Y and vz@gm_Z accumulate
                # onto the vx@gm_X region (rows 32:48 of the X column block),
                # so svd needs no vector-engine adds at all
                for h in range(n):
                    b = b0 + h
                    nc.tensor.matmul(
                        ps_agg[32:48, 512 * h + 128:512 * h + 256],
                        feat2[:, FD * h + 64:FD * h + 80], gm_blk(b, 2),
                        start=False, stop=False, skip_group_check=True)
                    nc.tensor.matmul(
                        ps_agg[32:48, 512 * h + 128:512 * h + 256],
                        feat2[:, FD * h + 96:FD * h + 112], gm_blk(b, 3),
                        start=False, stop=True, skip_group_check=True)
                # copies, split across Act/DVE by unit parity
                e1, e2 = (0, 1) if g % 2 == 0 else (1, 0)

                def cp(e, dst, srcp):
                    if e == 0:
                        nc.scalar.copy(dst, srcp)
                    else:
                        nc.vector.tensor_copy(dst, srcp)

                # ta: all features aggregated with plain adjacency A (pads = 0)
                ta = work.tile([FD, TB * 128], BF16, tag="ta")
                # stb rows: s@Ay(0:32) s@Az(32:64) s@Ax(64:96) svd(96:112)
                stb = work.tile([FD, TB * 128], BF16, tag="stb")
                cp(e1, g_dst(ta[:, 0:128], n), g_src(ps_agg, 0, FD, 0, n))
                cp(e2, g_dst(stb[64:112, 0:128], n), g_src(ps_agg, 0, 48, 128, n))
                cp(e1, g_dst(stb[0:32, 0:128], n), g_src(ps_agg, 0, 32, 256, n))
                cp(e2, g_dst(stb[32:64, 0:128], n), g_src(ps_agg, 0, 32, 384, n))
                ps_h = pp_h.tile([128, TB * FD], F32, tag="psh")
                wl_a = wta_t[:, FD * (l - 1):FD * l]
                wl_b = wtb_t[:, FD * (l - 1):FD * l]
                for h in range(n):
                    nc.tensor.matmul(ps_h[:, FD * h:FD * h + FD],
                                     ta[:, 128 * h:128 * h + 128], wl_a,
                                     start=True, stop=False)
                    nc.tensor.matmul(ps_h[:, FD * h:FD * h + FD],
                                     stb[:, 128 * h:128 * h + 128], wl_b,
                                     start=False, stop=True)
                pool = f3p if l == 2 else f2p
                featn = pool.tile([128, TB * FD], BF16, tag="f3" if l == 2 else "f2")
                w = FD * n
                nc.vector.scalar_tensor_tensor(
                    featn[:, 0:w], ps_h[:, 0:w], 0.0, feat2[:, 0:w],
                    AluOpType.max, AluOpType.add)
                return featn

            # phase-sequential: every group through L1, then L2, then L3 —
            # each phase is NG independent chains so engines stay saturated
            f1s = [emit_l1(g) for g in range(NG)]
            f2s = [emit_layer(1, g, f1s[g]) for g in range(NG)]
            f3s = [emit_layer(2, g, f2s[g]) for g in range(NG)]

            # sum-pool all graphs from the retained layer-3 features
            ps_pool = pp_agg.tile([FD, GPC], F32, tag="agg")
            for g in range(NG):
                b0, n = grp_blocks(g)
                for h in range(n):
                    b = b0 + h
                    nc.tensor.matmul(ps_pool[0:FD, 4 * b:4 * b + 4],
                                     f3s[g][:, FD * h:FD * h + FD], poolm_t[:],
                                     start=True, stop=True)

            # --- readout MLP: relu(x @ Wr1 + br1) @ Wr2 + br2, feature-major ---
            xfm = outp.tile([FD, GPC], BF16, tag="xfm")
            nc.vector.tensor_copy(xfm[:], ps_pool[:])
            ps_t1 = pp_agg.tile([128, GPC], F32, tag="agg")
            ps_t2 = pp_agg.tile([128, GPC], F32, tag="agg")
            nc.tensor.matmul(ps_t1[:], wr1_t[:, 0:128], xfm[:], start=True, stop=True)
            nc.tensor.matmul(ps_t2[:], wr1_t[:, 128:256], xfm[:], start=True, stop=True)
            hid1 = outp.tile([128, GPC], BF16, tag="hid1")
            hid2 = outp.tile([128, GPC], BF16, tag="hid2")
            nc.vector.tensor_scalar(hid1[:], ps_t1[:], br1_t[:, 0:1], 0.0,
                                    AluOpType.add, AluOpType.max)
            nc.vector.tensor_scalar(hid2[:], ps_t2[:], br1_t[:, 1:2], 0.0,
                                    AluOpType.add, AluOpType.max)
            ps_o = pp_agg.tile([LAT, GPC], F32, tag="agg")
            nc.tensor.matmul(ps_o[:], wr2_t[:, 0:128], hid1[:], start=True, stop=False)
            nc.tensor.matmul(ps_o[:], wr2_t[:, 128:256], hid2[:], start=False, stop=True)
            out_sb = outp.tile([LAT, GPC], F32, tag="out")
            nc.vector.tensor_scalar(out_sb[:], ps_o[:], br2_t[:], None, AluOpType.add)
            nc.sync.dma_start(out_ap[:], out_sb[:])

    nc.compile()
    return nc


def kernel(pos, emb, W_s2n, W1, W2, W3, W4, Ws, Wv, Wr1, br1, Wr2, br2,
           z, batch, edge_index, num_graphs):
    pos = np.asarray(pos, dtype=np.float32)
    z = np.asarray(z)
    emb = np.asarray(emb, dtype=np.float32)
    W_s2n = np.asarray(W_s2n, dtype=np.float32)
    W1 = np.asarray(W1, dtype=np.float32); W2 = np.asarray(W2, dtype=np.float32)
    W3 = np.asarray(W3, dtype=np.float32); W4 = np.asarray(W4, dtype=np.float32)
    Ws = np.asarray(Ws, dtype=np.float32); Wv = np.asarray(Wv, dtype=np.float32)
    Wr1 = np.asarray(Wr1, dtype=np.float32); br1 = np.asarray(br1, dtype=np.float32)
    Wr2 = np.asarray(Wr2, dtype=np.float32); br2 = np.asarray(br2, dtype=np.float32)

    # host prep: embedding lookup folded with input linear
    EW = (emb @ W_s2n) * np.float32(1.0 / np.sqrt(S_MUL))     # [100, 32]
    s0 = EW[z]                                                # [N, 32]

    # masked adjacency + spherical harmonics: gm[b, src, (type, dst)]
    pos_g = pos.reshape(B, NA, 3)
    diff = pos_g[:, None, :, :] - pos_g[:, :, None, :]        # [B, s, d, c] = pos[d]-pos[s]
    d2 = (diff * diff).sum(-1)
    mask = ((d2 <= 25.0) & (d2 > 0.0)).astype(np.float32)
    with np.errstate(divide="ignore", invalid="ignore"):
        inv_r = np.float32(np.sqrt(3.0)) / np.sqrt(d2)
    inv_r[~np.isfinite(inv_r)] = 0.0
    sh = diff * (mask * inv_r)[..., None]                     # [B, s, d, 3]
    NB4 = B // GPB
    Tall = np.empty((4, NB4, GPB, NA, NA), np.float32)
    Tall[0] = mask.reshape(NB4, GPB, NA, NA)
    for c in range(3):
        Tall[1 + c] = sh[..., c].reshape(NB4, GPB, NA, NA)
    TT = np.zeros((NB4, GPB, NA, 4, GPB, NA), np.float32)
    for g in range(GPB):
        TT[:, g, :, :, g, :] = Tall[:, :, g].transpose(1, 2, 0, 3)
    gm_all = TT.reshape(NB4, 128, 512)

    # folded tensor-product + linear weights
    cs = C_SCALAR * np.float32(1.0 / np.sqrt(S_MUL))
    csb = C_SCALAR * np.float32(INV_SQRT3 / np.sqrt(S_MUL))
    cv = C_VECTOR * np.float32(INV_SQRT3 / np.sqrt(V_MUL))
    Wa = [cs * (W1[l] @ Ws[l]) for l in range(3)]     # [32, 32]
    Wb = [csb * (W4[l] @ Ws[l]) for l in range(3)]    # [16, 32]
    Wc = [cv * (W2[l] @ Wv[l]) for l in range(3)]     # [32, 16]
    Wd = [cv * (W3[l] @ Wv[l]) for l in range(3)]     # [16, 16]

    # layer-1 transform applied on host (s0 is host-prepped anyway)
    s0w = np.concatenate([s0 @ Wa[0], s0 @ Wc[0]], axis=1)    # [N, 48]

    # feature/psum row layout: s(0:32) vx(32:48) pad vy(64:80) pad vz(96:112)
    # stb rows: [s@Ay(0:32), s@Az(32:64), s@Ax(64:96), svd(96:112)]
    wta = np.zeros((FD, 2 * FD), np.float32)
    wtb = np.zeros((FD, 2 * FD), np.float32)
    for l in (1, 2):
        o = FD * (l - 1)
        wta[0:32, o:o + 32] = Wa[l]
        for c in range(3):
            r = 32 * (c + 1)
            wta[r:r + 16, o + r:o + r + 16] = Wd[l]
        wtb[0:32, o + 64:o + 80] = Wc[l]      # s@Ay -> vy
        wtb[32:64, o + 96:o + 112] = Wc[l]    # s@Az -> vz
        wtb[64:96, o + 32:o + 48] = Wc[l]     # s@Ax -> vx
        wtb[96:112, o:o + 32] = Wb[l]         # svd  -> s

    poolm = np.zeros((128, GPB), np.float32)
    for g in range(GPB):
        poolm[g * NA:(g + 1) * NA, g] = 1.0

    # readout weights: v rows at 32*(c+1)+u map to original 32+3u+c
    wr1p = np.zeros((FD, HID), np.float32)
    wr1p[0:32] = Wr1[0:32]
    for c in range(3):
        for u in range(V_MUL):
            wr1p[32 * (c + 1) + u] = Wr1[32 + 3 * u + c]
    wr2p = np.zeros((128, HID), np.float32)
    wr2p[:, 0:128] = Wr2[0:128]
    wr2p[:, 128:256] = Wr2[128:256]
    br1t = br1.reshape(2, 128).T.copy()               # [128, 2]
    br2t = br2.reshape(LAT, 1)

    if "nc" not in _CACHE:
        _CACHE["nc"] = _build_program()
    nc = _CACHE["nc"]

    consts = dict(
        wta=wta.astype(BF16NP), wtb=wtb.astype(BF16NP),
        poolm=poolm.astype(BF16NP), wr1=wr1p.astype(BF16NP), wr2=wr2p.astype(BF16NP),
        br1=np.ascontiguousarray(br1t), br2=br2t,
    )
    in_maps = []
    for c in range(NCORES):
        gm_core = np.ascontiguousarray(
            gm_all[c * NBLK:(c + 1) * NBLK].transpose(1, 0, 2)
        ).reshape(128, NBLK * 512).astype(BF16NP)
        s0_core = np.ascontiguousarray(
            s0[c * NPC:(c + 1) * NPC].reshape(NBLK, 128, S_MUL).transpose(1, 0, 2)
        ).reshape(128, NBLK * S_MUL).astype(BF16NP)
        s0w_core = np.ascontiguousarray(
            s0w[c * NPC:(c + 1) * NPC].reshape(NBLK, 128, 48).transpose(1, 0, 2)
        ).reshape(128, NBLK * 48).astype(BF16NP)
        m = dict(consts)
        for k in range(GCH):
            m[f"gm{k}"] = np.ascontiguousarray(
                gm_core[:, k * BPCH * 512:(k + 1) * BPCH * 512])
        m["s0"] = s0_core
        m["s0w"] = s0w_core
        in_maps.append(m)

    res = run_bass_kernel_spmd(nc, in_maps, core_ids=list(range(NCORES)))
    out = np.empty((B, LAT), np.float32)
    for c in range(NCORES):
        out[c * GPC:(c + 1) * GPC] = res.results[c]["outfm"].T
    return out
